# revision 6
# baseline (speedup 1.0000x reference)
"""HGT kernel for 8 Trainium2 NeuronCores — fully on-device pipeline.

Design (transfer-minimal: the axon tunnel to the devices is ~70MB/s, so the
whole network runs on device; host only preprocesses indices and weights):
  - Nodes are degree-sorted and dealt round-robin across cores (sorted rank i
    -> core i%8, slot i//8), so every core's tile t covers the same degree
    band and one compile-time K_t fits all cores (identical SPMD NEFF).
  - Per core, a Y table [8832, 640] holds (kta|ktb|vta|vtb|q) for its 2560
    user rows + 6272 item rows, with the relation transforms A_k (scaled by
    p_rel/sqrt(D)) and A_v folded into the kqv weights as block-diagonal
    factors on the host.
  - Programs (each its own NEFF, chained through device-resident jax arrays;
    one jit may contain only a single bass_exec, so glue like all_gather and
    zeros-creation runs in separate pure-XLA jits):
      P1: input proj + relu + folded kqv  -> Y, XT          [row-sharded]
      AG: lax.all_gather(Y)               -> Yfull           [XLA jit]
      P2: edge phase (gather, masked segment softmax, weighted sum) -> OUT
          [dst-sharded; gathers k and v rows from Yfull viewed as
           [70656*5, 128] via per-edge indices idx=(row*5+colblock); the v
           gather reuses the same indices with element_offset=2*128]
      P3: gelu -> out linear -> sigmoid-gated skip -> relu -> next kqv
      P4: same head + shared final linear -> FINT [64, 8832]
  - Padding slots point at SENT=12801 (ktb column block of an item row,
    which is identically zero), and are masked to -1e30 before the softmax.
"""

import sys
import numpy as np

sys.path.insert(0, "/opt/trn_rl_repo")

H, D = 8, 16
HID = H * D
NU, NI = 20000, 50000
L = 2
NC = 8
P = 128
RU, RI = 2560, 6272          # per-core padded user/item rows
RALL = RU + RI               # 8832
NFULL = NC * RALL            # 70656
SENT = (RU) * 5 + 1          # 12801: ktb block of core-0 item row 0 (always 0)
INV_SQRT_D = 1.0 / np.sqrt(np.float32(D))

_LAST_HW_NS = None
_HW_NS_TOTAL = 0


# ---------------------------------------------------------------------------
# Host preprocessing
# ---------------------------------------------------------------------------

def _node_assign(deg):
    """degree-sort nodes; sorted rank i -> core i%8, slot i//8.
    Returns (order, core_of, slot_of): order[i] = node id at rank i."""
    order = np.argsort(deg, kind="stable")
    n = deg.shape[0]
    core_of = np.empty(n, np.int32)
    slot_of = np.empty(n, np.int32)
    ranks = np.arange(n)
    core_of[order] = (ranks % NC).astype(np.int32)
    slot_of[order] = (ranks // NC).astype(np.int32)
    return order, core_of, slot_of


def _edge_tables(dst_core, dst_slot, gidx, n_tiles, rows_pc):
    """Build per-core flat slot tables.

    dst_core/dst_slot: per-edge destination (core, slot). gidx: per-edge
    gather index into the [NFULL*5, 128] view. Returns (K_t list, flat int32
    [NC, TOT] table, tile offsets)."""
    E = gidx.shape[0]
    # position of each edge within its destination's list
    key = (dst_core.astype(np.int64) * rows_pc + dst_slot).astype(np.int64)
    order = np.argsort(key, kind="stable")
    ks = key[order]
    grp_start = np.zeros(E, np.int64)
    new_grp = np.ones(E, bool)
    new_grp[1:] = ks[1:] != ks[:-1]
    idx_of_start = np.nonzero(new_grp)[0]
    grp_start[idx_of_start] = idx_of_start
    grp_start = np.maximum.accumulate(np.where(new_grp, np.arange(E), 0))
    pos_sorted = np.arange(E) - grp_start
    pos = np.empty(E, np.int64)
    pos[order] = pos_sorted

    deg_pc = np.zeros((NC, rows_pc), np.int64)
    np.add.at(deg_pc, (dst_core, dst_slot), 1)
    # per-tile K shared across cores
    Kt = []
    for t in range(n_tiles):
        sl = slice(t * P, (t + 1) * P)
        Kt.append(int(deg_pc[:, sl].max()))
    offs = np.zeros(n_tiles + 1, np.int64)
    for t in range(n_tiles):
        offs[t + 1] = offs[t] + P * Kt[t]
    tot = int(offs[-1])
    tab = np.full((NC, tot), SENT, np.int32)
    t_of_slot = dst_slot // P
    p_of_slot = dst_slot % P
    kt_arr = np.asarray(Kt, np.int64)
    flat = offs[t_of_slot] + p_of_slot.astype(np.int64) * kt_arr[t_of_slot] + pos
    tab[dst_core, flat] = gidx.astype(np.int32)
    return Kt, tab, offs


def _blockdiag(blocks):
    out = np.zeros((HID, HID), dtype=np.float32)
    for h in range(H):
        out[h * D:(h + 1) * D, h * D:(h + 1) * D] = blocks[h]
    return out


def _sigmoid(x):
    return 1.0 / (1.0 + np.exp(-np.float64(x)))


def _preprocess(inp):
    """Everything host-side: permutations, edge tables, folded weights."""
    pre = {}
    e_ui = (np.asarray(inp["edge_src_ui"]), np.asarray(inp["edge_dst_ui"]))
    e_iu = (np.asarray(inp["edge_src_iu"]), np.asarray(inp["edge_dst_iu"]))
    e_uu = (np.asarray(inp["edge_src_uu"]), np.asarray(inp["edge_dst_uu"]))

    deg_u = np.bincount(e_iu[1], minlength=NU) + np.bincount(e_uu[1], minlength=NU)
    deg_i = np.bincount(e_ui[1], minlength=NI)
    pre["ord_u"], cu, su = _node_assign(deg_u)
    pre["ord_i"], ci, si = _node_assign(deg_i)

    # flat Y row of each node: user u -> core*RALL + slot ; item -> +RU
    urow = cu.astype(np.int64) * RALL + su
    irow = ci.astype(np.int64) * RALL + RU + si

    # user-dst aggregation: rel iu (item src, kta=col0) + rel uu (user src, ktb=col1)
    dstc = np.concatenate([cu[e_iu[1]], cu[e_uu[1]]])
    dsts = np.concatenate([su[e_iu[1]], su[e_uu[1]]])
    gidx = np.concatenate([irow[e_iu[0]] * 5 + 0, urow[e_uu[0]] * 5 + 1])
    pre["Ku"], pre["tab_u"], _ = _edge_tables(dstc, dsts, gidx, RU // P, RU)

    # item-dst aggregation: rel ui (user src, kta=col0)
    pre["Ki"], pre["tab_i"], _ = _edge_tables(
        ci[e_ui[1]], si[e_ui[1]], urow[e_ui[0]] * 5 + 0, RI // P, RI)

    # permuted x, transposed, per-core concat on axis 0
    x_user = np.asarray(inp["x_user"], np.float32)
    x_item = np.asarray(inp["x_item"], np.float32)
    xuT = np.zeros((NC, 128, RU), np.float32)
    xiT = np.zeros((NC, 64, RI), np.float32)
    xuT[cu, :, su] = x_user            # fancy index: rows to (core, :, slot)
    xiT[ci, :, si] = x_item
    pre["xuT"] = xuT.reshape(NC * 128, RU)
    pre["xiT"] = xiT.reshape(NC * 64, RI)

    # ---- fold weights ----
    A_k = np.asarray(inp["A_k"], np.float32)
    A_v = np.asarray(inp["A_v"], np.float32)
    p_rel = np.asarray(inp["p_rel"], np.float32)
    W = {}
    for l in range(L):
        Wk_u, Wq_u, Wv_u = np.split(np.asarray(inp["W_kqv_user"][l], np.float32), 3, axis=1)
        bk_u, bq_u, bv_u = np.split(np.asarray(inp["b_kqv_user"][l], np.float32), 3)
        Wk_i, Wq_i, Wv_i = np.split(np.asarray(inp["W_kqv_item"][l], np.float32), 3, axis=1)
        bk_i, bq_i, bv_i = np.split(np.asarray(inp["b_kqv_item"][l], np.float32), 3)

        def bk(r):
            return _blockdiag(A_k[l, r] * (p_rel[l, r] * INV_SQRT_D)[:, None, None])

        Bk0, Bk1, Bk2 = bk(0), bk(1), bk(2)
        Bv0, Bv1, Bv2 = (_blockdiag(A_v[l, r]) for r in range(3))
        # user cols: kta=k@Bk0, ktb=k@Bk2, vta=v@Bv0, vtb=v@Bv2, q
        W[f"WBU{l}"] = np.concatenate(
            [Wk_u @ Bk0, Wk_u @ Bk2, Wv_u @ Bv0, Wv_u @ Bv2, Wq_u], axis=1)
        W[f"BBU{l}"] = np.stack(
            [bk_u @ Bk0, bk_u @ Bk2, bv_u @ Bv0, bv_u @ Bv2, bq_u], axis=1)  # [128,5]
        # item cols: kta=k@Bk1, ktb=0, vta=v@Bv1, vtb=0, q
        Z = np.zeros((HID, HID), np.float32)
        W[f"WBI{l}"] = np.concatenate([Wk_i @ Bk1, Z, Wv_i @ Bv1, Z, Wq_i], axis=1)
        W[f"BBI{l}"] = np.stack(
            [bk_i @ Bk1, np.zeros(HID, np.float32), bv_i @ Bv1,
             np.zeros(HID, np.float32), bq_i], axis=1)
        g_u = np.float32(_sigmoid(inp["skip_user"][l]))
        g_i = np.float32(_sigmoid(inp["skip_item"][l]))
        W[f"GWOU{l}"] = g_u * np.asarray(inp["W_out_user"][l], np.float32)
        W[f"GBOU{l}"] = g_u * np.asarray(inp["b_out_user"][l], np.float32)
        W[f"GWOI{l}"] = g_i * np.asarray(inp["W_out_item"][l], np.float32)
        W[f"GBOI{l}"] = g_i * np.asarray(inp["b_out_item"][l], np.float32)
        W[f"cu{l}"] = float(1.0 - g_u)
        W[f"ci{l}"] = float(1.0 - g_i)
    Wlin = np.zeros((128, 128), np.float32)
    Wlin[:, :64] = np.asarray(inp["W_lin"], np.float32)
    W["WLIN"] = Wlin
    blin = np.zeros(128, np.float32)
    blin[:64] = np.asarray(inp["b_lin"], np.float32)
    W["BLIN"] = blin
    W["WINU"] = np.asarray(inp["W_in_user"], np.float32)
    W["BINU"] = np.asarray(inp["b_in_user"], np.float32)
    W["WINI"] = np.asarray(inp["W_in_item"], np.float32)
    W["BINI"] = np.asarray(inp["b_in_item"], np.float32)
    pre["W"] = W
    return pre


# ---------------------------------------------------------------------------
# Numpy emulation of the device pipeline (for validation / fallback)
# ---------------------------------------------------------------------------

def _np_edge_phase(Yfull, Ylocal_all, Klist, tab, row0):
    """Per-core edge phase, all cores at once. Returns OUT rows [NC, ntiles*128, 128]."""
    Yv = Yfull.reshape(-1, 128)      # [NFULL*5, 128]
    ntiles = len(Klist)
    out = np.zeros((NC, ntiles * P, HID), np.float32)
    for c in range(NC):
        off = 0
        for t in range(ntiles):
            K = Klist[t]
            if K == 0:
                continue
            idx = tab[c, off:off + P * K].reshape(P, K)
            off += P * K
            kg = Yv[idx]                       # [128, K, 128]
            vg = Yv[idx + 2]                   # element_offset 2 blocks
            q = Ylocal_all[c, row0 + t * P: row0 + (t + 1) * P, 512:640]
            s = (kg.reshape(P, K, H, D) * q.reshape(P, 1, H, D)).sum(-1)  # [128,K,H]
            s = s - 1e30 * (idx == SENT)[:, :, None]
            m = s.max(axis=1, keepdims=True)
            e = np.exp(s - m)
            den = e.sum(axis=1, keepdims=True)
            alpha = e / den
            o = (vg.reshape(P, K, H, D) * alpha[..., None]).sum(axis=1)
            out[c, t * P:(t + 1) * P] = o.reshape(P, HID)
    return out


def _np_pipeline(pre):
    """Numpy emulation of P1->P2->P3->P2->P4. Returns FINT [NC, 64, RALL]."""
    W = pre["W"]
    xuT = pre["xuT"].reshape(NC, 128, RU)
    xiT = pre["xiT"].reshape(NC, 64, RI)
    from scipy.special import erf

    def gelu(x):
        return 0.5 * x * (1.0 + erf(x / np.sqrt(2.0))).astype(np.float32)

    XT = np.zeros((NC, 128, RALL), np.float32)
    for c in range(NC):
        XT[c, :, :RU] = np.maximum(W["WINU"].T @ xuT[c] + W["BINU"][:, None], 0.0)
        XT[c, :, RU:] = np.maximum(W["WINI"].T @ xiT[c] + W["BINI"][:, None], 0.0)

    for l in range(L):
        Y = np.zeros((NC, RALL, 640), np.float32)
        # bias layout: BB [128, 5] -> col block b gets BB[:, b]
        for c in range(NC):
            bb_u = np.concatenate([W[f"BBU{l}"][:, b] for b in range(5)])
            bb_i = np.concatenate([W[f"BBI{l}"][:, b] for b in range(5)])
            Y[c, :RU] = XT[c, :, :RU].T @ W[f"WBU{l}"] + bb_u[None, :]
            Y[c, RU:] = XT[c, :, RU:].T @ W[f"WBI{l}"] + bb_i[None, :]
        Yfull = Y.reshape(NFULL, 640)
        out_u = _np_edge_phase(Yfull, Y, pre["Ku"], pre["tab_u"], 0)
        out_i = _np_edge_phase(Yfull, Y, pre["Ki"], pre["tab_i"], RU)
        XT2 = np.zeros_like(XT)
        for c in range(NC):
            aTu = W[f"GWOU{l}"].T @ gelu(out_u[c]).T + W[f"GBOU{l}"][:, None]
            aTi = W[f"GWOI{l}"].T @ gelu(out_i[c]).T + W[f"GBOI{l}"][:, None]
            XT2[c, :, :RU] = np.maximum(aTu + W[f"cu{l}"] * XT[c, :, :RU], 0.0)
            XT2[c, :, RU:] = np.maximum(aTi + W[f"ci{l}"] * XT[c, :, RU:], 0.0)
        XT = XT2
    FINT = np.zeros((NC, 64, RALL), np.float32)
    for c in range(NC):
        FINT[c] = (W["WLIN"].T @ XT[c] + W["BLIN"][:, None])[:64]
    return FINT


def _assemble(pre, FINT):
    """FINT [NC, 64, RALL] -> full output [NU+NI, 64] in original order."""
    out = np.zeros((NU + NI, 64), np.float32)
    un = FINT[:, :, :NU // NC]            # [NC, 64, 2500]
    A = un.transpose(2, 0, 1).reshape(NU, 64)   # rank i = 8*slot + core
    out[pre["ord_u"]] = A
    it = FINT[:, :, RU:RU + NI // NC]
    Bm = it.transpose(2, 0, 1).reshape(NI, 64)
    out[NU + np.asarray(pre["ord_i"])] = Bm
    return out


def kernel_numpy(**inp):
    pre = _preprocess(inp)
    return _assemble(pre, _np_pipeline(pre))


# ---------------------------------------------------------------------------
# Bass programs
# ---------------------------------------------------------------------------

def _slabs(n, w=512):
    out = []
    o = 0
    while o < n:
        out.append((o, min(w, n - o)))
        o += min(w, n - o)
    return out


def _build_P1():
    import concourse.bacc as bacc
    import concourse.mybir as mybir
    import concourse.tile as tile

    nc = bacc.Bacc("TRN2", target_bir_lowering=False, debug=False)
    xuT = nc.dram_tensor("xuT", [128, RU], mybir.dt.float32, kind="ExternalInput")
    xiT = nc.dram_tensor("xiT", [64, RI], mybir.dt.float32, kind="ExternalInput")
    WINU = nc.dram_tensor("WINU", [128, 128], mybir.dt.float32, kind="ExternalInput")
    BINU = nc.dram_tensor("BINU", [128, 1], mybir.dt.float32, kind="ExternalInput")
    WINI = nc.dram_tensor("WINI", [64, 128], mybir.dt.float32, kind="ExternalInput")
    BINI = nc.dram_tensor("BINI", [128, 1], mybir.dt.float32, kind="ExternalInput")
    WBU = nc.dram_tensor("WBU", [128, 640], mybir.dt.float32, kind="ExternalInput")
    BBU = nc.dram_tensor("BBU", [128, 5], mybir.dt.float32, kind="ExternalInput")
    WBI = nc.dram_tensor("WBI", [128, 640], mybir.dt.float32, kind="ExternalInput")
    BBI = nc.dram_tensor("BBI", [128, 5], mybir.dt.float32, kind="ExternalInput")
    Y = nc.dram_tensor("Y", [RALL, 640], mybir.dt.float32, kind="ExternalOutput")
    XT = nc.dram_tensor("XT", [128, RALL], mybir.dt.float32, kind="ExternalOutput")

    AF = mybir.ActivationFunctionType
    with tile.TileContext(nc) as tc:
        with (
            tc.tile_pool(name="w", bufs=1) as wp,
            tc.tile_pool(name="x", bufs=3) as xp,
            tc.tile_pool(name="y", bufs=3) as yp,
            tc.tile_pool(name="ps", bufs=2, space="PSUM") as pp,
        ):
            w_in_u = wp.tile([128, 128], mybir.dt.float32, tag="wiu")
            nc.sync.dma_start(out=w_in_u[:], in_=WINU[:, :])
            w_in_i = wp.tile([64, 128], mybir.dt.float32, tag="wii")
            nc.sync.dma_start(out=w_in_i[:], in_=WINI[:, :])
            b_in_u = wp.tile([128, 1], mybir.dt.float32, tag="biu")
            nc.sync.dma_start(out=b_in_u[:], in_=BINU[:, :])
            b_in_i = wp.tile([128, 1], mybir.dt.float32, tag="bii")
            nc.sync.dma_start(out=b_in_i[:], in_=BINI[:, :])
            wb_u = wp.tile([128, 640], mybir.dt.float32, tag="wbu")
            nc.sync.dma_start(out=wb_u[:], in_=WBU[:, :])
            wb_i = wp.tile([128, 640], mybir.dt.float32, tag="wbi")
            nc.sync.dma_start(out=wb_i[:], in_=WBI[:, :])
            bb_u = wp.tile([128, 5], mybir.dt.float32, tag="bbu")
            nc.sync.dma_start(out=bb_u[:], in_=BBU[:, :])
            bb_i = wp.tile([128, 5], mybir.dt.float32, tag="bbi")
            nc.sync.dma_start(out=bb_i[:], in_=BBI[:, :])

            for (xT_d, w_in, b_in, wb, bb, kin, r0, rows) in (
                (xuT, w_in_u, b_in_u, wb_u, bb_u, 128, 0, RU),
                (xiT, w_in_i, b_in_i, wb_i, bb_i, 64, RU, RI),
            ):
                for (s, w) in _slabs(rows):
                    x_sb = xp.tile([kin, 512], mybir.dt.float32, tag="x")
                    nc.sync.dma_start(out=x_sb[:, :w], in_=xT_d[:, s:s + w])
                    psx = pp.tile([128, 512], mybir.dt.float32, space="PSUM", tag="px")
                    nc.tensor.matmul(out=psx[:, :w], lhsT=w_in[:, :], rhs=x_sb[:, :w],
                                     start=True, stop=True)
                    xh = xp.tile([128, 512], mybir.dt.float32, tag="xh")
                    nc.scalar.activation(out=xh[:, :w], in_=psx[:, :w], func=AF.Relu,
                                         bias=b_in[:, 0:1], scale=1.0)
                    nc.sync.dma_start(out=XT[:, r0 + s: r0 + s + w], in_=xh[:, :w])
                    y_sb = yp.tile([128, 5, 512], mybir.dt.float32, tag="y")
                    for b in range(5):
                        psy = pp.tile([128, 512], mybir.dt.float32, space="PSUM",
                                      tag=f"py{b % 2}")
                        nc.tensor.matmul(out=psy[:, :w], lhsT=wb[:, b * 128:(b + 1) * 128],
                                         rhs=xh[:, :w], start=True, stop=True)
                        nc.scalar.activation(out=y_sb[:, b, :w], in_=psy[:, :w],
                                             func=AF.Identity, bias=bb[:, b:b + 1],
                                             scale=1.0)
                    for b in range(5):
                        nc.sync.dma_start(
                            out=Y[r0 + s: r0 + s + w, b * 128:(b + 1) * 128]
                            .rearrange("r p -> p r"),
                            in_=y_sb[:, b, :w])
    nc.compile()
    return nc


def _build_P2(Klist_u, Klist_i, tot):
    import concourse.bacc as bacc
    import concourse.mybir as mybir
    import concourse.tile as tile
    from concourse import bass

    nc = bacc.Bacc("TRN2", target_bir_lowering=False, debug=False)
    Yfull = nc.dram_tensor("Yfull", [NFULL, 640], mybir.dt.float32, kind="ExternalInput")
    Yloc = nc.dram_tensor("Yloc", [RALL, 640], mybir.dt.float32, kind="ExternalInput")
    TAB = nc.dram_tensor("TAB", [tot], mybir.dt.int32, kind="ExternalInput")
    OUT = nc.dram_tensor("OUT", [RALL, 128], mybir.dt.float32, kind="ExternalOutput")

    AF = mybir.ActivationFunctionType
    ALU = mybir.AluOpType
    Yv = Yfull[:, :].rearrange("a (b c) -> (a b) c", b=5, c=128)

    with tile.TileContext(nc) as tc:
        with (
            tc.tile_pool(name="ix", bufs=2) as ixp,
            tc.tile_pool(name="kv", bufs=2) as kvp,
            tc.tile_pool(name="sm", bufs=2) as smp,
            tc.tile_pool(name="o", bufs=2) as op,
        ):
            off = 0
            tlist = [(t, K, t * P) for t, K in enumerate(Klist_u)] + \
                    [(len(Klist_u) + t, K, RU + t * P) for t, K in enumerate(Klist_i)]
            for (tt, K, row0) in tlist:
                if K == 0:
                    oz = op.tile([128, 128], mybir.dt.float32, tag="oz")
                    nc.vector.memset(oz[:], 0.0)
                    nc.sync.dma_start(out=OUT[row0:row0 + P, :], in_=oz[:])
                    continue
                idx = ixp.tile([128, K], mybir.dt.int32, tag="idx")
                nc.sync.dma_start(
                    out=idx[:],
                    in_=TAB[off:off + P * K].rearrange("(p k) -> p k", p=128, k=K))
                off += P * K
                # one gather per slot: 3 contiguous 128-blocks from row idx
                # (k at block 0, v at block +2; middle block unused)
                gath = kvp.tile([128, K * 384], mybir.dt.float32, tag="kg")
                for k in range(K):
                    nc.gpsimd.indirect_dma_start(
                        out=gath[:, k * 384:(k + 1) * 384], out_offset=None, in_=Yv,
                        in_offset=bass.IndirectOffsetOnAxis(ap=idx[:, k:k + 1], axis=0))
                g3 = gath[:].rearrange("p (k c) -> p k c", k=K, c=384)
                kg4 = g3[:, :, 0:128].rearrange("p k (h d) -> p k h d", h=H, d=D)
                vg4 = g3[:, :, 256:384].rearrange("p k (h d) -> p k h d", h=H, d=D)
                q = smp.tile([128, 128], mybir.dt.float32, tag="q")
                nc.sync.dma_start(out=q[:], in_=Yloc[row0:row0 + P, 512:640].rearrange(
                    "r c -> r c"))
                # qk product in-place into kg
                qb = q[:].rearrange("p (h d) -> p h d", h=H, d=D) \
                    .unsqueeze(1).broadcast_to([128, K, H, D])
                nc.vector.tensor_tensor(out=kg4, in0=kg4, in1=qb, op=ALU.mult)
                # scores [128, K, H]
                s = smp.tile([128, K * H], mybir.dt.float32, tag="s")
                s3 = s[:].rearrange("p (k h) -> p k h", k=K, h=H)
                nc.vector.tensor_reduce(out=s3, in_=kg4, axis=mybir.AxisListType.X,
                                        op=ALU.add)
                # mask pad slots
                mskf = smp.tile([128, K], mybir.dt.float32, tag="mf")
                nc.vector.tensor_copy(out=mskf[:], in_=idx[:])
                nc.vector.tensor_scalar(out=mskf[:], in0=mskf[:],
                                        scalar1=float(SENT), scalar2=1e30,
                                        op0=ALU.is_equal, op1=ALU.mult)
                mb = mskf[:].unsqueeze(2).broadcast_to([128, K, H])
                nc.vector.tensor_tensor(out=s3, in0=s3, in1=mb, op=ALU.subtract)
                # softmax over K per head
                m = smp.tile([128, H], mybir.dt.float32, tag="m")
                sT = s3.transpose([0, 2, 1])       # [p, h, k]
                nc.vector.tensor_reduce(out=m[:], in_=sT, axis=mybir.AxisListType.X,
                                        op=ALU.max)
                mb2 = m[:].unsqueeze(1).broadcast_to([128, K, H])
                nc.vector.tensor_tensor(out=s3, in0=s3, in1=mb2, op=ALU.subtract)
                nc.scalar.activation(out=s[:], in_=s[:], func=AF.Exp)
                den = smp.tile([128, H], mybir.dt.float32, tag="den")
                nc.vector.tensor_reduce(out=den[:], in_=sT, axis=mybir.AxisListType.X,
                                        op=ALU.add)
                rden = smp.tile([128, H], mybir.dt.float32, tag="rden")
                nc.vector.reciprocal(out=rden[:], in_=den[:])
                rb = rden[:].unsqueeze(1).broadcast_to([128, K, H])
                nc.vector.tensor_tensor(out=s3, in0=s3, in1=rb, op=ALU.mult)
                # weighted values: vg *= alpha ; reduce over K
                ab = s3.unsqueeze(3).broadcast_to([128, K, H, D])
                nc.vector.tensor_tensor(out=vg4, in0=vg4, in1=ab, op=ALU.mult)
                o = op.tile([128, 128], mybir.dt.float32, tag="o")
                o3 = o[:].rearrange("p (h d) -> p h d", h=H, d=D)
                vgr = vg4.transpose([0, 2, 3, 1])   # [p, h, d, k]
                nc.vector.tensor_reduce(out=o3, in_=vgr, axis=mybir.AxisListType.X,
                                        op=ALU.add)
                nc.sync.dma_start(out=OUT[row0:row0 + P, :], in_=o[:])
    nc.compile()
    return nc


def _build_P34(lidx, final, consts):
    """P3 (next-layer kqv) or P4 (final linear). consts: dict with folded weights
    as compile-time values for (1-g); weights come in as tensors."""
    import concourse.bacc as bacc
    import concourse.mybir as mybir
    import concourse.tile as tile
    from concourse.masks import make_identity

    nc = bacc.Bacc("TRN2", target_bir_lowering=False, debug=False)
    OUT = nc.dram_tensor("OUT", [RALL, 128], mybir.dt.float32, kind="ExternalInput")
    XTP = nc.dram_tensor("XTP", [128, RALL], mybir.dt.float32, kind="ExternalInput")
    GWOU = nc.dram_tensor("GWOU", [128, 128], mybir.dt.float32, kind="ExternalInput")
    GBOU = nc.dram_tensor("GBOU", [128, 1], mybir.dt.float32, kind="ExternalInput")
    GWOI = nc.dram_tensor("GWOI", [128, 128], mybir.dt.float32, kind="ExternalInput")
    GBOI = nc.dram_tensor("GBOI", [128, 1], mybir.dt.float32, kind="ExternalInput")
    if final:
        WLIN = nc.dram_tensor("WLIN", [128, 128], mybir.dt.float32, kind="ExternalInput")
        BLIN = nc.dram_tensor("BLIN", [128, 1], mybir.dt.float32, kind="ExternalInput")
        FINT = nc.dram_tensor("FINT", [64, RALL], mybir.dt.float32, kind="ExternalOutput")
    else:
        WBU = nc.dram_tensor("WBU", [128, 640], mybir.dt.float32, kind="ExternalInput")
        BBU = nc.dram_tensor("BBU", [128, 5], mybir.dt.float32, kind="ExternalInput")
        WBI = nc.dram_tensor("WBI", [128, 640], mybir.dt.float32, kind="ExternalInput")
        BBI = nc.dram_tensor("BBI", [128, 5], mybir.dt.float32, kind="ExternalInput")
        Y = nc.dram_tensor("Y", [RALL, 640], mybir.dt.float32, kind="ExternalOutput")
        XT2 = nc.dram_tensor("XT2", [128, RALL], mybir.dt.float32, kind="ExternalOutput")

    cu, ci = consts["cu"], consts["ci"]
    AF = mybir.ActivationFunctionType
    ALU = mybir.AluOpType
    with tile.TileContext(nc) as tc:
        with (
            tc.tile_pool(name="w", bufs=1) as wp,
            tc.tile_pool(name="x", bufs=3) as xp,
            tc.tile_pool(name="y", bufs=3) as yp,
            tc.tile_pool(name="ps", bufs=2, space="PSUM") as pp,
        ):
            ident = wp.tile([128, 128], mybir.dt.float32, tag="id")
            make_identity(nc, ident[:])
            gwo_u = wp.tile([128, 128], mybir.dt.float32, tag="gwu")
            nc.sync.dma_start(out=gwo_u[:], in_=GWOU[:, :])
            gwo_i = wp.tile([128, 128], mybir.dt.float32, tag="gwi")
            nc.sync.dma_start(out=gwo_i[:], in_=GWOI[:, :])
            gbo_u = wp.tile([128, 1], mybir.dt.float32, tag="gbu")
            nc.sync.dma_start(out=gbo_u[:], in_=GBOU[:, :])
            gbo_i = wp.tile([128, 1], mybir.dt.float32, tag="gbi")
            nc.sync.dma_start(out=gbo_i[:], in_=GBOI[:, :])
            if final:
                wlin = wp.tile([128, 128], mybir.dt.float32, tag="wl")
                nc.sync.dma_start(out=wlin[:], in_=WLIN[:, :])
                blin = wp.tile([128, 1], mybir.dt.float32, tag="bl")
                nc.sync.dma_start(out=blin[:], in_=BLIN[:, :])
            else:
                wb_u = wp.tile([128, 640], mybir.dt.float32, tag="wbu")
                nc.sync.dma_start(out=wb_u[:], in_=WBU[:, :])
                wb_i = wp.tile([128, 640], mybir.dt.float32, tag="wbi")
                nc.sync.dma_start(out=wb_i[:], in_=WBI[:, :])
                bb_u = wp.tile([128, 5], mybir.dt.float32, tag="bbu")
                nc.sync.dma_start(out=bb_u[:], in_=BBU[:, :])
                bb_i = wp.tile([128, 5], mybir.dt.float32, tag="bbi")
                nc.sync.dma_start(out=bb_i[:], in_=BBI[:, :])

            for (gwo, gbo, cc, wb, bb, r0, rows) in (
                (gwo_u, gbo_u, cu, "u", "u", 0, RU),
                (gwo_i, gbo_i, ci, "i", "i", RU, RI),
            ):
                for (s, w) in _slabs(rows):
                    gt = xp.tile([128, 512], mybir.dt.float32, tag="gt")
                    nsub = (w + 127) // 128
                    for sub in range(nsub):
                        ww = min(128, w - sub * 128)
                        ot = xp.tile([128, 128], mybir.dt.float32, tag="ot")
                        nc.sync.dma_start(
                            out=ot[:ww, :],
                            in_=OUT[r0 + s + sub * 128: r0 + s + sub * 128 + ww, :])
                        pst = pp.tile([128, 128], mybir.dt.float32, space="PSUM",
                                      tag="pt")
                        nc.tensor.transpose(out=pst[:, :ww], in_=ot[:ww, :],
                                            identity=ident[:])
                        # gelu(x) ~ 0.5 x (1 + tanh(0.79788456 x + 0.03567741 x^3))
                        gu = xp.tile([128, 128], mybir.dt.float32, tag="gu")
                        nc.scalar.square(out=gu[:, :ww], in_=pst[:, :ww])
                        nc.vector.tensor_scalar(
                            out=gu[:, :ww], in0=gu[:, :ww], scalar1=0.035677408,
                            scalar2=0.7978845608, op0=ALU.mult, op1=ALU.add)
                        nc.vector.tensor_tensor(out=gu[:, :ww], in0=gu[:, :ww],
                                                in1=pst[:, :ww], op=ALU.mult)
                        nc.scalar.activation(out=gu[:, :ww], in_=gu[:, :ww],
                                             func=AF.Tanh)
                        nc.vector.tensor_scalar(
                            out=gu[:, :ww], in0=gu[:, :ww], scalar1=1.0,
                            scalar2=0.5, op0=ALU.add, op1=ALU.mult)
                        nc.vector.tensor_tensor(
                            out=gt[:, sub * 128: sub * 128 + ww], in0=gu[:, :ww],
                            in1=pst[:, :ww], op=ALU.mult)
                    psa = pp.tile([128, 512], mybir.dt.float32, space="PSUM", tag="pa")
                    nc.tensor.matmul(out=psa[:, :w], lhsT=gwo[:, :], rhs=gt[:, :w],
                                     start=True, stop=True)
                    aT = xp.tile([128, 512], mybir.dt.float32, tag="aT")
                    nc.scalar.activation(out=aT[:, :w], in_=psa[:, :w],
                                         func=AF.Identity, bias=gbo[:, 0:1], scale=1.0)
                    xprev = xp.tile([128, 512], mybir.dt.float32, tag="xp")
                    nc.sync.dma_start(out=xprev[:, :w], in_=XTP[:, r0 + s: r0 + s + w])
                    nc.vector.tensor_scalar_mul(out=xprev[:, :w],
                                                in0=xprev[:, :w], scalar1=cc)
                    nc.vector.tensor_tensor(out=aT[:, :w], in0=aT[:, :w],
                                            in1=xprev[:, :w], op=ALU.add)
                    xh = xp.tile([128, 512], mybir.dt.float32, tag="xh")
                    nc.scalar.activation(out=xh[:, :w], in_=aT[:, :w], func=AF.Relu)
                    if final:
                        psf = pp.tile([128, 512], mybir.dt.float32, space="PSUM",
                                      tag="pf")
                        nc.tensor.matmul(out=psf[:, :w], lhsT=wlin[:, :], rhs=xh[:, :w],
                                         start=True, stop=True)
                        fT = yp.tile([128, 512], mybir.dt.float32, tag="fT")
                        nc.scalar.activation(out=fT[:, :w], in_=psf[:, :w],
                                             func=AF.Identity, bias=blin[:, 0:1],
                                             scale=1.0)
                        nc.sync.dma_start(out=FINT[:, r0 + s: r0 + s + w],
                                          in_=fT[:64, :w])
                    else:
                        nc.sync.dma_start(out=XT2[:, r0 + s: r0 + s + w],
                                          in_=xh[:, :w])
                        wbt = wb_u if wb == "u" else wb_i
                        bbt = bb_u if bb == "u" else bb_i
                        y_sb = yp.tile([128, 5, 512], mybir.dt.float32, tag="y")
                        for b in range(5):
                            psy = pp.tile([128, 512], mybir.dt.float32, space="PSUM",
                                          tag=f"py{b % 2}")
                            nc.tensor.matmul(out=psy[:, :w],
                                             lhsT=wbt[:, b * 128:(b + 1) * 128],
                                             rhs=xh[:, :w], start=True, stop=True)
                            nc.scalar.activation(out=y_sb[:, b, :w], in_=psy[:, :w],
                                                 func=AF.Identity, bias=bbt[:, b:b + 1],
                                                 scale=1.0)
                        for b in range(5):
                            nc.sync.dma_start(
                                out=Y[r0 + s: r0 + s + w, b * 128:(b + 1) * 128]
                                .rearrange("r p -> p r"),
                                in_=y_sb[:, b, :w])
    nc.compile()
    return nc


# ---------------------------------------------------------------------------
# Launcher: persistent jits, device-resident flow
# ---------------------------------------------------------------------------

class _Prog:
    def __init__(self, nc):
        import jax
        from concourse import mybir
        from concourse.bass2jax import (_bass_exec_p, partition_id_tensor,
                                        install_neuronx_cc_hook)
        install_neuronx_cc_hook()
        self.nc = nc
        self.partition_name = (nc.partition_id_tensor.name
                               if nc.partition_id_tensor else None)
        self.in_names, self.out_names, self.out_avals = [], [], []
        self._in_avals = {}
        for alloc in nc.m.functions[0].allocations:
            if not isinstance(alloc, mybir.MemoryLocationSet):
                continue
            name = alloc.memorylocations[0].name
            if alloc.kind == "ExternalInput":
                if name != self.partition_name:
                    self.in_names.append(name)
                    self._in_avals[name] = jax.core.ShapedArray(
                        tuple(alloc.tensor_shape), mybir.dt.np(alloc.dtype))
            elif alloc.kind == "ExternalOutput":
                self.out_names.append(name)
                self.out_avals.append(jax.core.ShapedArray(
                    tuple(alloc.tensor_shape), mybir.dt.np(alloc.dtype)))
        self._jit = None

    def _make(self, mesh):
        import jax
        import numpy as np
        from jax.sharding import PartitionSpec
        from jax.experimental.shard_map import shard_map
        from concourse.bass2jax import _bass_exec_p, partition_id_tensor
        n_params = len(self.in_names)
        n_outs = len(self.out_names)
        all_in = list(self.in_names) + list(self.out_names)
        pname = self.partition_name
        if pname is not None:
            all_in.append(pname)
        out_avals = tuple(self.out_avals)
        out_names = tuple(self.out_names)
        nc = self.nc

        def _body(*args):
            operands = list(args)
            if pname is not None:
                operands.append(partition_id_tensor())
            outs = _bass_exec_p.bind(
                *operands,
                out_avals=out_avals,
                in_names=tuple(all_in),
                out_names=out_names,
                lowering_input_output_aliases=(),
                sim_require_finite=False,
                sim_require_nnan=False,
                nc=nc,
            )
            return tuple(outs)

        in_specs = (PartitionSpec("core"),) * (n_params + n_outs)
        out_specs = (PartitionSpec("core"),) * n_outs
        self._jit = jax.jit(
            shard_map(_body, mesh=mesh, in_specs=in_specs, out_specs=out_specs,
                      check_rep=False),
            keep_unused=True)

    def compile_ahead(self, mesh, shard):
        import jax
        if self._jit is None:
            self._make(mesh)
        specs = []
        for name in self.in_names:
            a = self._in_avals[name]
            specs.append(jax.ShapeDtypeStruct(
                (len(mesh.devices) * a.shape[0],) + tuple(a.shape[1:]), a.dtype,
                sharding=shard))
        for a in self.out_avals:
            specs.append(jax.ShapeDtypeStruct(
                (len(mesh.devices) * a.shape[0],) + tuple(a.shape[1:]), a.dtype,
                sharding=shard))
        self._compiled = self._jit.lower(*specs).compile()

    def __call__(self, mesh, zeros_fn, **kw):
        if self._jit is None:
            self._make(mesh)
        ops = [kw[n] for n in self.in_names]
        zeros = [zeros_fn(tuple(a.shape), a.dtype) for a in self.out_avals]
        fn = getattr(self, "_compiled", None) or self._jit
        outs = fn(*ops, *zeros)
        return dict(zip(self.out_names, outs))


_cache = {}


def kernel(**inp):
    import os, time
    global _LAST_HW_NS, _HW_NS_TOTAL
    t_all = time.time()
    _dbg = os.environ.get("HGT_DEBUG")
    _tp = [time.time()]

    def _t(tag):
        if _dbg:
            now = time.time()
            print(f"[hgt] {tag}: {now - _tp[0]:.3f}s", flush=True)
            _tp[0] = now

    import jax
    import jax.numpy as jnp
    from jax.sharding import Mesh, PartitionSpec, NamedSharding
    from jax.experimental.shard_map import shard_map

    pre = _preprocess(inp)
    W = pre["W"]
    _t("preprocess")

    if "mesh" not in _cache:
        devices = jax.devices()[:NC]
        _cache["mesh"] = Mesh(np.asarray(devices), ("core",))
    mesh = _cache["mesh"]
    shard = NamedSharding(mesh, PartitionSpec("core"))

    tot_u = pre["tab_u"].shape[1]
    tot_i = pre["tab_i"].shape[1]
    key = ("progs", tuple(pre["Ku"]), tuple(pre["Ki"]),
           W["cu0"], W["ci0"], W["cu1"], W["ci1"])
    futs = None
    if key not in _cache:
        from concurrent.futures import ThreadPoolExecutor
        _cache.clear()
        _cache["mesh"] = mesh
        ex = ThreadPoolExecutor(max_workers=6)

        def mk(builder, *a):
            p = _Prog(builder(*a))
            p.compile_ahead(mesh, shard)
            return p

        def mk_glue():
            def _ag(y):
                return jax.lax.all_gather(y, "core", axis=0, tiled=True)

            agj = jax.jit(shard_map(_ag, mesh=mesh,
                                    in_specs=(PartitionSpec("core"),),
                                    out_specs=PartitionSpec("core"),
                                    check_rep=False))
            spec = jax.ShapeDtypeStruct((NC * RALL, 640), np.float32,
                                        sharding=shard)
            agc = agj.lower(spec).compile()
            zjit = jax.jit(jnp.zeros, static_argnums=(0, 1), out_shardings=shard)
            zcache = {}
            for sh, dt in (((RALL, 640), np.float32), ((128, RALL), np.float32),
                           ((RALL, 128), np.float32), ((64, RALL), np.float32)):
                g = (NC * sh[0],) + sh[1:]
                zcache[g] = zjit(g, np.dtype(dt))

            def _zeros(shape, dtype):
                gshape = (NC * shape[0],) + tuple(shape[1:])
                if gshape not in zcache:
                    zcache[gshape] = zjit(gshape, dtype)
                return zcache[gshape]

            return agc, _zeros

        futs = dict(
            p1=ex.submit(mk, _build_P1),
            p2=ex.submit(mk, _build_P2, pre["Ku"], pre["Ki"], tot_u + tot_i),
            p3=ex.submit(mk, _build_P34, 0, False,
                         {"cu": W["cu0"], "ci": W["ci0"]}),
            p4=ex.submit(mk, _build_P34, 1, True,
                         {"cu": W["cu1"], "ci": W["ci1"]}),
            glue=ex.submit(mk_glue),
        )
        _t("threads spawned")

    def rep(a):
        """replicate a per-core tensor: concat 8 copies on axis 0."""
        return np.concatenate([a] * NC, axis=0)

    dev = lambda a: jax.device_put(np.ascontiguousarray(a), shard)

    t0 = time.time()
    args1 = dict(
        xuT=dev(pre["xuT"]), xiT=dev(pre["xiT"]),
        WINU=dev(rep(W["WINU"])), BINU=dev(rep(W["BINU"].reshape(128, 1))),
        WINI=dev(rep(W["WINI"])), BINI=dev(rep(W["BINI"].reshape(128, 1))),
    )
    tab = np.concatenate([
        np.concatenate([pre["tab_u"][c], pre["tab_i"][c]]) for c in range(NC)])
    TAB = dev(tab)
    wb = {}
    for l in range(L):
        wb[f"WBU{l}"] = dev(rep(W[f"WBU{l}"]))
        wb[f"BBU{l}"] = dev(rep(W[f"BBU{l}"]))
        wb[f"WBI{l}"] = dev(rep(W[f"WBI{l}"]))
        wb[f"BBI{l}"] = dev(rep(W[f"BBI{l}"]))
        wb[f"GWOU{l}"] = dev(rep(W[f"GWOU{l}"]))
        wb[f"GBOU{l}"] = dev(rep(W[f"GBOU{l}"].reshape(128, 1)))
        wb[f"GWOI{l}"] = dev(rep(W[f"GWOI{l}"]))
        wb[f"GBOI{l}"] = dev(rep(W[f"GBOI{l}"].reshape(128, 1)))
    wlin = dev(rep(W["WLIN"]))
    blin = dev(rep(W["BLIN"].reshape(128, 1)))
    _t("uploads")
    if futs is not None:
        _cache[key] = (futs["p1"].result(), futs["p2"].result(),
                       futs["p3"].result(), futs["p4"].result(),
                       *futs["glue"].result())
    p1, p2, p3, p4, agj, _zeros = _cache[key]
    _t("program builds+compiles joined")

    r1 = p1(mesh, _zeros, **args1, WBU=wb["WBU0"], BBU=wb["BBU0"],
            WBI=wb["WBI0"], BBI=wb["BBI0"])
    Y, XT = r1["Y"], r1["XT"]
    _t("P1 dispatch")
    Yfull = agj(Y)
    _t("AG1 dispatch")
    OUT = p2(mesh, _zeros, Yfull=Yfull, Yloc=Y, TAB=TAB)["OUT"]
    _t("P2a dispatch")
    r3 = p3(mesh, _zeros, OUT=OUT, XTP=XT,
            GWOU=wb["GWOU0"], GBOU=wb["GBOU0"], GWOI=wb["GWOI0"], GBOI=wb["GBOI0"],
            WBU=wb["WBU1"], BBU=wb["BBU1"], WBI=wb["WBI1"], BBI=wb["BBI1"])
    Y2, XT2 = r3["Y"], r3["XT2"]
    _t("P3 dispatch")
    Yfull2 = agj(Y2)
    OUT2 = p2(mesh, _zeros, Yfull=Yfull2, Yloc=Y2, TAB=TAB)["OUT"]
    _t("AG2+P2b dispatch")
    r4 = p4(mesh, _zeros, OUT=OUT2, XTP=XT2,
            GWOU=wb["GWOU1"], GBOU=wb["GBOU1"], GWOI=wb["GWOI1"], GBOI=wb["GBOI1"],
            WLIN=wlin, BLIN=blin)
    _t("P4 dispatch")
    FINT = np.asarray(r4["FINT"]).reshape(NC, 64, RALL)
    _t("FINT download/block")
    dt_ns = int((time.time() - t0) * 1e9)
    _LAST_HW_NS = dt_ns
    _HW_NS_TOTAL += dt_ns

    out = _assemble(pre, FINT)
    _t("assemble")
    return out


# revision 7
# speedup vs baseline: 4.6669x; 4.6669x over previous
"""HGT kernel for 8 Trainium2 NeuronCores — fully on-device pipeline.

Design (transfer-minimal: the axon tunnel to the devices is ~70MB/s, so the
whole network runs on device; host only preprocesses indices and weights):
  - Nodes are degree-sorted and dealt round-robin across cores (sorted rank i
    -> core i%8, slot i//8), so every core's tile t covers the same degree
    band and one compile-time K_t fits all cores (identical SPMD NEFF).
  - Per core, a Y table [8832, 640] holds (kta|ktb|vta|vtb|q) for its 2560
    user rows + 6272 item rows, with the relation transforms A_k (scaled by
    p_rel/sqrt(D)) and A_v folded into the kqv weights as block-diagonal
    factors on the host.
  - Programs (each its own NEFF, chained through device-resident jax arrays;
    one jit may contain only a single bass_exec, so glue like all_gather and
    zeros-creation runs in separate pure-XLA jits):
      P1: input proj + relu + folded kqv  -> Y, XT          [row-sharded]
      AG: lax.all_gather(Y)               -> Yfull           [XLA jit]
      P2: edge phase (gather, masked segment softmax, weighted sum) -> OUT
          [dst-sharded; gathers k and v rows from Yfull viewed as
           [70656*5, 128] via per-edge indices idx=(row*5+colblock); the v
           gather reuses the same indices with element_offset=2*128]
      P3: gelu -> out linear -> sigmoid-gated skip -> relu -> next kqv
      P4: same head + shared final linear -> FINT [64, 8832]
  - Padding slots point at SENT=12801 (ktb column block of an item row,
    which is identically zero), and are masked to -1e30 before the softmax.
"""

import sys
import numpy as np

sys.path.insert(0, "/opt/trn_rl_repo")

H, D = 8, 16
HID = H * D
NU, NI = 20000, 50000
L = 2
NC = 8
P = 128
RU, RI = 2560, 6272          # per-core padded user/item rows
RALL = RU + RI               # 8832
NFULL = NC * RALL            # 70656
SENT = (RU) * 5 + 1          # 12801: ktb block of core-0 item row 0 (always 0)
INV_SQRT_D = 1.0 / np.sqrt(np.float32(D))

_LAST_HW_NS = None
_HW_NS_TOTAL = 0


# ---------------------------------------------------------------------------
# Host preprocessing
# ---------------------------------------------------------------------------

def _node_assign(deg):
    """degree-sort nodes; sorted rank i -> core i%8, slot i//8.
    Returns (order, core_of, slot_of): order[i] = node id at rank i."""
    order = np.argsort(deg, kind="stable")
    n = deg.shape[0]
    core_of = np.empty(n, np.int32)
    slot_of = np.empty(n, np.int32)
    ranks = np.arange(n)
    core_of[order] = (ranks % NC).astype(np.int32)
    slot_of[order] = (ranks // NC).astype(np.int32)
    return order, core_of, slot_of


def _edge_tables(dst_core, dst_slot, gidx, n_tiles, rows_pc):
    """Build per-core flat slot tables.

    dst_core/dst_slot: per-edge destination (core, slot). gidx: per-edge
    gather index into the [NFULL*5, 128] view. Returns (K_t list, flat int32
    [NC, TOT] table, tile offsets)."""
    E = gidx.shape[0]
    # position of each edge within its destination's list
    key = (dst_core.astype(np.int64) * rows_pc + dst_slot).astype(np.int64)
    order = np.argsort(key, kind="stable")
    ks = key[order]
    grp_start = np.zeros(E, np.int64)
    new_grp = np.ones(E, bool)
    new_grp[1:] = ks[1:] != ks[:-1]
    idx_of_start = np.nonzero(new_grp)[0]
    grp_start[idx_of_start] = idx_of_start
    grp_start = np.maximum.accumulate(np.where(new_grp, np.arange(E), 0))
    pos_sorted = np.arange(E) - grp_start
    pos = np.empty(E, np.int64)
    pos[order] = pos_sorted

    deg_pc = np.zeros((NC, rows_pc), np.int64)
    np.add.at(deg_pc, (dst_core, dst_slot), 1)
    # per-tile K shared across cores
    Kt = []
    for t in range(n_tiles):
        sl = slice(t * P, (t + 1) * P)
        Kt.append(int(deg_pc[:, sl].max()))
    offs = np.zeros(n_tiles + 1, np.int64)
    for t in range(n_tiles):
        offs[t + 1] = offs[t] + P * Kt[t]
    tot = int(offs[-1])
    tab = np.full((NC, tot), SENT, np.int32)
    t_of_slot = dst_slot // P
    p_of_slot = dst_slot % P
    kt_arr = np.asarray(Kt, np.int64)
    flat = offs[t_of_slot] + p_of_slot.astype(np.int64) * kt_arr[t_of_slot] + pos
    tab[dst_core, flat] = gidx.astype(np.int32)
    return Kt, tab, offs


def _blockdiag(blocks):
    out = np.zeros((HID, HID), dtype=np.float32)
    for h in range(H):
        out[h * D:(h + 1) * D, h * D:(h + 1) * D] = blocks[h]
    return out


def _sigmoid(x):
    return 1.0 / (1.0 + np.exp(-np.float64(x)))


def _preprocess(inp):
    """Everything host-side: permutations, edge tables, folded weights."""
    pre = {}
    e_ui = (np.asarray(inp["edge_src_ui"]), np.asarray(inp["edge_dst_ui"]))
    e_iu = (np.asarray(inp["edge_src_iu"]), np.asarray(inp["edge_dst_iu"]))
    e_uu = (np.asarray(inp["edge_src_uu"]), np.asarray(inp["edge_dst_uu"]))

    deg_u = np.bincount(e_iu[1], minlength=NU) + np.bincount(e_uu[1], minlength=NU)
    deg_i = np.bincount(e_ui[1], minlength=NI)
    pre["ord_u"], cu, su = _node_assign(deg_u)
    pre["ord_i"], ci, si = _node_assign(deg_i)

    # flat Y row of each node: user u -> core*RALL + slot ; item -> +RU
    urow = cu.astype(np.int64) * RALL + su
    irow = ci.astype(np.int64) * RALL + RU + si

    # user-dst aggregation: rel iu (item src, kta=col0) + rel uu (user src, ktb=col1)
    dstc = np.concatenate([cu[e_iu[1]], cu[e_uu[1]]])
    dsts = np.concatenate([su[e_iu[1]], su[e_uu[1]]])
    gidx = np.concatenate([irow[e_iu[0]] * 5 + 0, urow[e_uu[0]] * 5 + 1])
    pre["Ku"], pre["tab_u"], _ = _edge_tables(dstc, dsts, gidx, RU // P, RU)

    # item-dst aggregation: rel ui (user src, kta=col0)
    pre["Ki"], pre["tab_i"], _ = _edge_tables(
        ci[e_ui[1]], si[e_ui[1]], urow[e_ui[0]] * 5 + 0, RI // P, RI)

    # permuted x, transposed, per-core concat on axis 0
    x_user = np.asarray(inp["x_user"], np.float32)
    x_item = np.asarray(inp["x_item"], np.float32)
    xuT = np.zeros((NC, 128, RU), np.float32)
    xiT = np.zeros((NC, 64, RI), np.float32)
    xuT[cu, :, su] = x_user            # fancy index: rows to (core, :, slot)
    xiT[ci, :, si] = x_item
    pre["xuT"] = xuT.reshape(NC * 128, RU)
    pre["xiT"] = xiT.reshape(NC * 64, RI)

    # ---- fold weights ----
    A_k = np.asarray(inp["A_k"], np.float32)
    A_v = np.asarray(inp["A_v"], np.float32)
    p_rel = np.asarray(inp["p_rel"], np.float32)
    W = {}
    for l in range(L):
        Wk_u, Wq_u, Wv_u = np.split(np.asarray(inp["W_kqv_user"][l], np.float32), 3, axis=1)
        bk_u, bq_u, bv_u = np.split(np.asarray(inp["b_kqv_user"][l], np.float32), 3)
        Wk_i, Wq_i, Wv_i = np.split(np.asarray(inp["W_kqv_item"][l], np.float32), 3, axis=1)
        bk_i, bq_i, bv_i = np.split(np.asarray(inp["b_kqv_item"][l], np.float32), 3)

        def bk(r):
            return _blockdiag(A_k[l, r] * (p_rel[l, r] * INV_SQRT_D)[:, None, None])

        Bk0, Bk1, Bk2 = bk(0), bk(1), bk(2)
        Bv0, Bv1, Bv2 = (_blockdiag(A_v[l, r]) for r in range(3))
        # user cols: kta=k@Bk0, ktb=k@Bk2, vta=v@Bv0, vtb=v@Bv2, q
        W[f"WBU{l}"] = np.concatenate(
            [Wk_u @ Bk0, Wk_u @ Bk2, Wv_u @ Bv0, Wv_u @ Bv2, Wq_u], axis=1)
        W[f"BBU{l}"] = np.stack(
            [bk_u @ Bk0, bk_u @ Bk2, bv_u @ Bv0, bv_u @ Bv2, bq_u], axis=1)  # [128,5]
        # item cols: kta=k@Bk1, ktb=0, vta=v@Bv1, vtb=0, q
        Z = np.zeros((HID, HID), np.float32)
        W[f"WBI{l}"] = np.concatenate([Wk_i @ Bk1, Z, Wv_i @ Bv1, Z, Wq_i], axis=1)
        W[f"BBI{l}"] = np.stack(
            [bk_i @ Bk1, np.zeros(HID, np.float32), bv_i @ Bv1,
             np.zeros(HID, np.float32), bq_i], axis=1)
        g_u = np.float32(_sigmoid(inp["skip_user"][l]))
        g_i = np.float32(_sigmoid(inp["skip_item"][l]))
        W[f"GWOU{l}"] = g_u * np.asarray(inp["W_out_user"][l], np.float32)
        W[f"GBOU{l}"] = g_u * np.asarray(inp["b_out_user"][l], np.float32)
        W[f"GWOI{l}"] = g_i * np.asarray(inp["W_out_item"][l], np.float32)
        W[f"GBOI{l}"] = g_i * np.asarray(inp["b_out_item"][l], np.float32)
        W[f"cu{l}"] = float(1.0 - g_u)
        W[f"ci{l}"] = float(1.0 - g_i)
    Wlin = np.zeros((128, 128), np.float32)
    Wlin[:, :64] = np.asarray(inp["W_lin"], np.float32)
    W["WLIN"] = Wlin
    blin = np.zeros(128, np.float32)
    blin[:64] = np.asarray(inp["b_lin"], np.float32)
    W["BLIN"] = blin
    W["WINU"] = np.asarray(inp["W_in_user"], np.float32)
    W["BINU"] = np.asarray(inp["b_in_user"], np.float32)
    W["WINI"] = np.asarray(inp["W_in_item"], np.float32)
    W["BINI"] = np.asarray(inp["b_in_item"], np.float32)
    pre["W"] = W
    return pre


# ---------------------------------------------------------------------------
# Numpy emulation of the device pipeline (for validation / fallback)
# ---------------------------------------------------------------------------

def _np_edge_phase(Yfull, Ylocal_all, Klist, tab, row0):
    """Per-core edge phase, all cores at once. Returns OUT rows [NC, ntiles*128, 128]."""
    Yv = Yfull.reshape(-1, 128)      # [NFULL*5, 128]
    ntiles = len(Klist)
    out = np.zeros((NC, ntiles * P, HID), np.float32)
    for c in range(NC):
        off = 0
        for t in range(ntiles):
            K = Klist[t]
            if K == 0:
                continue
            idx = tab[c, off:off + P * K].reshape(P, K)
            off += P * K
            kg = Yv[idx]                       # [128, K, 128]
            vg = Yv[idx + 2]                   # element_offset 2 blocks
            q = Ylocal_all[c, row0 + t * P: row0 + (t + 1) * P, 512:640]
            s = (kg.reshape(P, K, H, D) * q.reshape(P, 1, H, D)).sum(-1)  # [128,K,H]
            s = s - 1e30 * (idx == SENT)[:, :, None]
            m = s.max(axis=1, keepdims=True)
            e = np.exp(s - m)
            den = e.sum(axis=1, keepdims=True)
            alpha = e / den
            o = (vg.reshape(P, K, H, D) * alpha[..., None]).sum(axis=1)
            out[c, t * P:(t + 1) * P] = o.reshape(P, HID)
    return out


def _np_pipeline(pre):
    """Numpy emulation of P1->P2->P3->P2->P4. Returns FINT [NC, 64, RALL]."""
    W = pre["W"]
    xuT = pre["xuT"].reshape(NC, 128, RU)
    xiT = pre["xiT"].reshape(NC, 64, RI)
    from scipy.special import erf

    def gelu(x):
        return 0.5 * x * (1.0 + erf(x / np.sqrt(2.0))).astype(np.float32)

    XT = np.zeros((NC, 128, RALL), np.float32)
    for c in range(NC):
        XT[c, :, :RU] = np.maximum(W["WINU"].T @ xuT[c] + W["BINU"][:, None], 0.0)
        XT[c, :, RU:] = np.maximum(W["WINI"].T @ xiT[c] + W["BINI"][:, None], 0.0)

    for l in range(L):
        Y = np.zeros((NC, RALL, 640), np.float32)
        # bias layout: BB [128, 5] -> col block b gets BB[:, b]
        for c in range(NC):
            bb_u = np.concatenate([W[f"BBU{l}"][:, b] for b in range(5)])
            bb_i = np.concatenate([W[f"BBI{l}"][:, b] for b in range(5)])
            Y[c, :RU] = XT[c, :, :RU].T @ W[f"WBU{l}"] + bb_u[None, :]
            Y[c, RU:] = XT[c, :, RU:].T @ W[f"WBI{l}"] + bb_i[None, :]
        Yfull = Y.reshape(NFULL, 640)
        out_u = _np_edge_phase(Yfull, Y, pre["Ku"], pre["tab_u"], 0)
        out_i = _np_edge_phase(Yfull, Y, pre["Ki"], pre["tab_i"], RU)
        XT2 = np.zeros_like(XT)
        for c in range(NC):
            aTu = W[f"GWOU{l}"].T @ gelu(out_u[c]).T + W[f"GBOU{l}"][:, None]
            aTi = W[f"GWOI{l}"].T @ gelu(out_i[c]).T + W[f"GBOI{l}"][:, None]
            XT2[c, :, :RU] = np.maximum(aTu + W[f"cu{l}"] * XT[c, :, :RU], 0.0)
            XT2[c, :, RU:] = np.maximum(aTi + W[f"ci{l}"] * XT[c, :, RU:], 0.0)
        XT = XT2
    FINT = np.zeros((NC, 64, RALL), np.float32)
    for c in range(NC):
        FINT[c] = (W["WLIN"].T @ XT[c] + W["BLIN"][:, None])[:64]
    return FINT


def _assemble(pre, FINT):
    """FINT [NC, 64, RALL] -> full output [NU+NI, 64] in original order."""
    out = np.zeros((NU + NI, 64), np.float32)
    un = FINT[:, :, :NU // NC]            # [NC, 64, 2500]
    A = un.transpose(2, 0, 1).reshape(NU, 64)   # rank i = 8*slot + core
    out[pre["ord_u"]] = A
    it = FINT[:, :, RU:RU + NI // NC]
    Bm = it.transpose(2, 0, 1).reshape(NI, 64)
    out[NU + np.asarray(pre["ord_i"])] = Bm
    return out


def kernel_numpy(**inp):
    pre = _preprocess(inp)
    return _assemble(pre, _np_pipeline(pre))


# ---------------------------------------------------------------------------
# Bass programs
# ---------------------------------------------------------------------------

def _slabs(n, w=512):
    out = []
    o = 0
    while o < n:
        out.append((o, min(w, n - o)))
        o += min(w, n - o)
    return out


def _build_P1():
    import concourse.bacc as bacc
    import concourse.mybir as mybir
    import concourse.tile as tile

    nc = bacc.Bacc("TRN2", target_bir_lowering=False, debug=False)
    xuT = nc.dram_tensor("xuT", [128, RU], mybir.dt.float32, kind="ExternalInput")
    xiT = nc.dram_tensor("xiT", [64, RI], mybir.dt.float32, kind="ExternalInput")
    WINU = nc.dram_tensor("WINU", [128, 128], mybir.dt.float32, kind="ExternalInput")
    BINU = nc.dram_tensor("BINU", [128, 1], mybir.dt.float32, kind="ExternalInput")
    WINI = nc.dram_tensor("WINI", [64, 128], mybir.dt.float32, kind="ExternalInput")
    BINI = nc.dram_tensor("BINI", [128, 1], mybir.dt.float32, kind="ExternalInput")
    WBU = nc.dram_tensor("WBU", [128, 640], mybir.dt.float32, kind="ExternalInput")
    BBU = nc.dram_tensor("BBU", [128, 5], mybir.dt.float32, kind="ExternalInput")
    WBI = nc.dram_tensor("WBI", [128, 640], mybir.dt.float32, kind="ExternalInput")
    BBI = nc.dram_tensor("BBI", [128, 5], mybir.dt.float32, kind="ExternalInput")
    Y = nc.dram_tensor("Y", [RALL, 640], mybir.dt.float32, kind="ExternalOutput")
    XT = nc.dram_tensor("XT", [128, RALL], mybir.dt.float32, kind="ExternalOutput")

    AF = mybir.ActivationFunctionType
    with tile.TileContext(nc) as tc:
        with (
            tc.tile_pool(name="w", bufs=1) as wp,
            tc.tile_pool(name="x", bufs=3) as xp,
            tc.tile_pool(name="y", bufs=3) as yp,
            tc.tile_pool(name="ps", bufs=2, space="PSUM") as pp,
        ):
            w_in_u = wp.tile([128, 128], mybir.dt.float32, tag="wiu")
            nc.sync.dma_start(out=w_in_u[:], in_=WINU[:, :])
            w_in_i = wp.tile([64, 128], mybir.dt.float32, tag="wii")
            nc.sync.dma_start(out=w_in_i[:], in_=WINI[:, :])
            b_in_u = wp.tile([128, 1], mybir.dt.float32, tag="biu")
            nc.sync.dma_start(out=b_in_u[:], in_=BINU[:, :])
            b_in_i = wp.tile([128, 1], mybir.dt.float32, tag="bii")
            nc.sync.dma_start(out=b_in_i[:], in_=BINI[:, :])
            wb_u = wp.tile([128, 640], mybir.dt.float32, tag="wbu")
            nc.sync.dma_start(out=wb_u[:], in_=WBU[:, :])
            wb_i = wp.tile([128, 640], mybir.dt.float32, tag="wbi")
            nc.sync.dma_start(out=wb_i[:], in_=WBI[:, :])
            bb_u = wp.tile([128, 5], mybir.dt.float32, tag="bbu")
            nc.sync.dma_start(out=bb_u[:], in_=BBU[:, :])
            bb_i = wp.tile([128, 5], mybir.dt.float32, tag="bbi")
            nc.sync.dma_start(out=bb_i[:], in_=BBI[:, :])

            for (xT_d, w_in, b_in, wb, bb, kin, r0, rows) in (
                (xuT, w_in_u, b_in_u, wb_u, bb_u, 128, 0, RU),
                (xiT, w_in_i, b_in_i, wb_i, bb_i, 64, RU, RI),
            ):
                for (s, w) in _slabs(rows):
                    x_sb = xp.tile([kin, 512], mybir.dt.float32, tag="x")
                    nc.sync.dma_start(out=x_sb[:, :w], in_=xT_d[:, s:s + w])
                    psx = pp.tile([128, 512], mybir.dt.float32, space="PSUM", tag="px")
                    nc.tensor.matmul(out=psx[:, :w], lhsT=w_in[:, :], rhs=x_sb[:, :w],
                                     start=True, stop=True)
                    xh = xp.tile([128, 512], mybir.dt.float32, tag="xh")
                    nc.scalar.activation(out=xh[:, :w], in_=psx[:, :w], func=AF.Relu,
                                         bias=b_in[:, 0:1], scale=1.0)
                    nc.sync.dma_start(out=XT[:, r0 + s: r0 + s + w], in_=xh[:, :w])
                    y_sb = yp.tile([128, 5, 512], mybir.dt.float32, tag="y")
                    for b in range(5):
                        psy = pp.tile([128, 512], mybir.dt.float32, space="PSUM",
                                      tag=f"py{b % 2}")
                        nc.tensor.matmul(out=psy[:, :w], lhsT=wb[:, b * 128:(b + 1) * 128],
                                         rhs=xh[:, :w], start=True, stop=True)
                        nc.scalar.activation(out=y_sb[:, b, :w], in_=psy[:, :w],
                                             func=AF.Identity, bias=bb[:, b:b + 1],
                                             scale=1.0)
                    for b in range(5):
                        nc.sync.dma_start(
                            out=Y[r0 + s: r0 + s + w, b * 128:(b + 1) * 128]
                            .rearrange("r p -> p r"),
                            in_=y_sb[:, b, :w])
    nc.compile()
    return nc


def _build_P2(Klist_u, Klist_i, tot):
    import concourse.bacc as bacc
    import concourse.mybir as mybir
    import concourse.tile as tile
    from concourse import bass

    nc = bacc.Bacc("TRN2", target_bir_lowering=False, debug=False)
    Yfull = nc.dram_tensor("Yfull", [NFULL, 640], mybir.dt.float32, kind="ExternalInput")
    Yloc = nc.dram_tensor("Yloc", [RALL, 640], mybir.dt.float32, kind="ExternalInput")
    TAB = nc.dram_tensor("TAB", [tot], mybir.dt.int32, kind="ExternalInput")
    OUT = nc.dram_tensor("OUT", [RALL, 128], mybir.dt.float32, kind="ExternalOutput")

    AF = mybir.ActivationFunctionType
    ALU = mybir.AluOpType
    Yv = Yfull[:, :].rearrange("a (b c) -> (a b) c", b=5, c=128)

    with tile.TileContext(nc) as tc:
        with (
            tc.tile_pool(name="ix", bufs=2) as ixp,
            tc.tile_pool(name="kv", bufs=2) as kvp,
            tc.tile_pool(name="sm", bufs=2) as smp,
            tc.tile_pool(name="o", bufs=2) as op,
        ):
            off = 0
            tlist = [(t, K, t * P) for t, K in enumerate(Klist_u)] + \
                    [(len(Klist_u) + t, K, RU + t * P) for t, K in enumerate(Klist_i)]
            for (tt, K, row0) in tlist:
                if K == 0:
                    oz = op.tile([128, 128], mybir.dt.float32, tag="oz")
                    nc.vector.memset(oz[:], 0.0)
                    nc.sync.dma_start(out=OUT[row0:row0 + P, :], in_=oz[:])
                    continue
                idx = ixp.tile([128, K], mybir.dt.int32, tag="idx")
                nc.sync.dma_start(
                    out=idx[:],
                    in_=TAB[off:off + P * K].rearrange("(p k) -> p k", p=128, k=K))
                off += P * K
                # one gather per slot: 3 contiguous 128-blocks from row idx
                # (k at block 0, v at block +2; middle block unused)
                gath = kvp.tile([128, K * 384], mybir.dt.float32, tag="kg")
                for k in range(K):
                    nc.gpsimd.indirect_dma_start(
                        out=gath[:, k * 384:(k + 1) * 384], out_offset=None, in_=Yv,
                        in_offset=bass.IndirectOffsetOnAxis(ap=idx[:, k:k + 1], axis=0))
                g3 = gath[:].rearrange("p (k c) -> p k c", k=K, c=384)
                kg4 = g3[:, :, 0:128].rearrange("p k (h d) -> p k h d", h=H, d=D)
                vg4 = g3[:, :, 256:384].rearrange("p k (h d) -> p k h d", h=H, d=D)
                q = smp.tile([128, 128], mybir.dt.float32, tag="q")
                nc.sync.dma_start(out=q[:], in_=Yloc[row0:row0 + P, 512:640].rearrange(
                    "r c -> r c"))
                # qk product in-place into kg
                qb = q[:].rearrange("p (h d) -> p h d", h=H, d=D) \
                    .unsqueeze(1).broadcast_to([128, K, H, D])
                nc.vector.tensor_tensor(out=kg4, in0=kg4, in1=qb, op=ALU.mult)
                # scores [128, K, H]
                s = smp.tile([128, K * H], mybir.dt.float32, tag="s")
                s3 = s[:].rearrange("p (k h) -> p k h", k=K, h=H)
                nc.vector.tensor_reduce(out=s3, in_=kg4, axis=mybir.AxisListType.X,
                                        op=ALU.add)
                # mask pad slots
                mskf = smp.tile([128, K], mybir.dt.float32, tag="mf")
                nc.vector.tensor_copy(out=mskf[:], in_=idx[:])
                nc.vector.tensor_scalar(out=mskf[:], in0=mskf[:],
                                        scalar1=float(SENT), scalar2=1e30,
                                        op0=ALU.is_equal, op1=ALU.mult)
                mb = mskf[:].unsqueeze(2).broadcast_to([128, K, H])
                nc.vector.tensor_tensor(out=s3, in0=s3, in1=mb, op=ALU.subtract)
                # softmax over K per head
                m = smp.tile([128, H], mybir.dt.float32, tag="m")
                sT = s3.transpose([0, 2, 1])       # [p, h, k]
                nc.vector.tensor_reduce(out=m[:], in_=sT, axis=mybir.AxisListType.X,
                                        op=ALU.max)
                mb2 = m[:].unsqueeze(1).broadcast_to([128, K, H])
                nc.vector.tensor_tensor(out=s3, in0=s3, in1=mb2, op=ALU.subtract)
                nc.scalar.activation(out=s[:], in_=s[:], func=AF.Exp)
                den = smp.tile([128, H], mybir.dt.float32, tag="den")
                nc.vector.tensor_reduce(out=den[:], in_=sT, axis=mybir.AxisListType.X,
                                        op=ALU.add)
                rden = smp.tile([128, H], mybir.dt.float32, tag="rden")
                nc.vector.reciprocal(out=rden[:], in_=den[:])
                rb = rden[:].unsqueeze(1).broadcast_to([128, K, H])
                nc.vector.tensor_tensor(out=s3, in0=s3, in1=rb, op=ALU.mult)
                # weighted values: vg *= alpha ; reduce over K
                ab = s3.unsqueeze(3).broadcast_to([128, K, H, D])
                nc.vector.tensor_tensor(out=vg4, in0=vg4, in1=ab, op=ALU.mult)
                o = op.tile([128, 128], mybir.dt.float32, tag="o")
                o3 = o[:].rearrange("p (h d) -> p h d", h=H, d=D)
                vgr = vg4.transpose([0, 2, 3, 1])   # [p, h, d, k]
                nc.vector.tensor_reduce(out=o3, in_=vgr, axis=mybir.AxisListType.X,
                                        op=ALU.add)
                nc.sync.dma_start(out=OUT[row0:row0 + P, :], in_=o[:])
    nc.compile()
    return nc


def _build_P34(lidx, final, consts):
    """P3 (next-layer kqv) or P4 (final linear). consts: dict with folded weights
    as compile-time values for (1-g); weights come in as tensors."""
    import concourse.bacc as bacc
    import concourse.mybir as mybir
    import concourse.tile as tile
    from concourse.masks import make_identity

    nc = bacc.Bacc("TRN2", target_bir_lowering=False, debug=False)
    OUT = nc.dram_tensor("OUT", [RALL, 128], mybir.dt.float32, kind="ExternalInput")
    XTP = nc.dram_tensor("XTP", [128, RALL], mybir.dt.float32, kind="ExternalInput")
    GWOU = nc.dram_tensor("GWOU", [128, 128], mybir.dt.float32, kind="ExternalInput")
    GBOU = nc.dram_tensor("GBOU", [128, 1], mybir.dt.float32, kind="ExternalInput")
    GWOI = nc.dram_tensor("GWOI", [128, 128], mybir.dt.float32, kind="ExternalInput")
    GBOI = nc.dram_tensor("GBOI", [128, 1], mybir.dt.float32, kind="ExternalInput")
    if final:
        WLIN = nc.dram_tensor("WLIN", [128, 128], mybir.dt.float32, kind="ExternalInput")
        BLIN = nc.dram_tensor("BLIN", [128, 1], mybir.dt.float32, kind="ExternalInput")
        FINT = nc.dram_tensor("FINT", [64, RALL], mybir.dt.float32, kind="ExternalOutput")
    else:
        WBU = nc.dram_tensor("WBU", [128, 640], mybir.dt.float32, kind="ExternalInput")
        BBU = nc.dram_tensor("BBU", [128, 5], mybir.dt.float32, kind="ExternalInput")
        WBI = nc.dram_tensor("WBI", [128, 640], mybir.dt.float32, kind="ExternalInput")
        BBI = nc.dram_tensor("BBI", [128, 5], mybir.dt.float32, kind="ExternalInput")
        Y = nc.dram_tensor("Y", [RALL, 640], mybir.dt.float32, kind="ExternalOutput")
        XT2 = nc.dram_tensor("XT2", [128, RALL], mybir.dt.float32, kind="ExternalOutput")

    cu, ci = consts["cu"], consts["ci"]
    AF = mybir.ActivationFunctionType
    ALU = mybir.AluOpType
    with tile.TileContext(nc) as tc:
        with (
            tc.tile_pool(name="w", bufs=1) as wp,
            tc.tile_pool(name="x", bufs=3) as xp,
            tc.tile_pool(name="y", bufs=3) as yp,
            tc.tile_pool(name="ps", bufs=2, space="PSUM") as pp,
        ):
            ident = wp.tile([128, 128], mybir.dt.float32, tag="id")
            make_identity(nc, ident[:])
            gwo_u = wp.tile([128, 128], mybir.dt.float32, tag="gwu")
            nc.sync.dma_start(out=gwo_u[:], in_=GWOU[:, :])
            gwo_i = wp.tile([128, 128], mybir.dt.float32, tag="gwi")
            nc.sync.dma_start(out=gwo_i[:], in_=GWOI[:, :])
            gbo_u = wp.tile([128, 1], mybir.dt.float32, tag="gbu")
            nc.sync.dma_start(out=gbo_u[:], in_=GBOU[:, :])
            gbo_i = wp.tile([128, 1], mybir.dt.float32, tag="gbi")
            nc.sync.dma_start(out=gbo_i[:], in_=GBOI[:, :])
            if final:
                wlin = wp.tile([128, 128], mybir.dt.float32, tag="wl")
                nc.sync.dma_start(out=wlin[:], in_=WLIN[:, :])
                blin = wp.tile([128, 1], mybir.dt.float32, tag="bl")
                nc.sync.dma_start(out=blin[:], in_=BLIN[:, :])
            else:
                wb_u = wp.tile([128, 640], mybir.dt.float32, tag="wbu")
                nc.sync.dma_start(out=wb_u[:], in_=WBU[:, :])
                wb_i = wp.tile([128, 640], mybir.dt.float32, tag="wbi")
                nc.sync.dma_start(out=wb_i[:], in_=WBI[:, :])
                bb_u = wp.tile([128, 5], mybir.dt.float32, tag="bbu")
                nc.sync.dma_start(out=bb_u[:], in_=BBU[:, :])
                bb_i = wp.tile([128, 5], mybir.dt.float32, tag="bbi")
                nc.sync.dma_start(out=bb_i[:], in_=BBI[:, :])

            for (gwo, gbo, cc, wb, bb, r0, rows) in (
                (gwo_u, gbo_u, cu, "u", "u", 0, RU),
                (gwo_i, gbo_i, ci, "i", "i", RU, RI),
            ):
                for (s, w) in _slabs(rows):
                    gt = xp.tile([128, 512], mybir.dt.float32, tag="gt")
                    nsub = (w + 127) // 128
                    for sub in range(nsub):
                        ww = min(128, w - sub * 128)
                        ot = xp.tile([128, 128], mybir.dt.float32, tag="ot")
                        nc.sync.dma_start(
                            out=ot[:ww, :],
                            in_=OUT[r0 + s + sub * 128: r0 + s + sub * 128 + ww, :])
                        pst = pp.tile([128, 128], mybir.dt.float32, space="PSUM",
                                      tag="pt")
                        nc.tensor.transpose(out=pst[:, :ww], in_=ot[:ww, :],
                                            identity=ident[:])
                        # gelu(x) ~ 0.5 x (1 + tanh(0.79788456 x + 0.03567741 x^3))
                        gu = xp.tile([128, 128], mybir.dt.float32, tag="gu")
                        nc.scalar.square(out=gu[:, :ww], in_=pst[:, :ww])
                        nc.vector.tensor_scalar(
                            out=gu[:, :ww], in0=gu[:, :ww], scalar1=0.035677408,
                            scalar2=0.7978845608, op0=ALU.mult, op1=ALU.add)
                        nc.vector.tensor_tensor(out=gu[:, :ww], in0=gu[:, :ww],
                                                in1=pst[:, :ww], op=ALU.mult)
                        nc.scalar.activation(out=gu[:, :ww], in_=gu[:, :ww],
                                             func=AF.Tanh)
                        nc.vector.tensor_scalar(
                            out=gu[:, :ww], in0=gu[:, :ww], scalar1=1.0,
                            scalar2=0.5, op0=ALU.add, op1=ALU.mult)
                        nc.vector.tensor_tensor(
                            out=gt[:, sub * 128: sub * 128 + ww], in0=gu[:, :ww],
                            in1=pst[:, :ww], op=ALU.mult)
                    psa = pp.tile([128, 512], mybir.dt.float32, space="PSUM", tag="pa")
                    nc.tensor.matmul(out=psa[:, :w], lhsT=gwo[:, :], rhs=gt[:, :w],
                                     start=True, stop=True)
                    aT = xp.tile([128, 512], mybir.dt.float32, tag="aT")
                    nc.scalar.activation(out=aT[:, :w], in_=psa[:, :w],
                                         func=AF.Identity, bias=gbo[:, 0:1], scale=1.0)
                    xprev = xp.tile([128, 512], mybir.dt.float32, tag="xp")
                    nc.sync.dma_start(out=xprev[:, :w], in_=XTP[:, r0 + s: r0 + s + w])
                    nc.vector.tensor_scalar_mul(out=xprev[:, :w],
                                                in0=xprev[:, :w], scalar1=cc)
                    nc.vector.tensor_tensor(out=aT[:, :w], in0=aT[:, :w],
                                            in1=xprev[:, :w], op=ALU.add)
                    xh = xp.tile([128, 512], mybir.dt.float32, tag="xh")
                    nc.scalar.activation(out=xh[:, :w], in_=aT[:, :w], func=AF.Relu)
                    if final:
                        psf = pp.tile([128, 512], mybir.dt.float32, space="PSUM",
                                      tag="pf")
                        nc.tensor.matmul(out=psf[:, :w], lhsT=wlin[:, :], rhs=xh[:, :w],
                                         start=True, stop=True)
                        fT = yp.tile([128, 512], mybir.dt.float32, tag="fT")
                        nc.scalar.activation(out=fT[:, :w], in_=psf[:, :w],
                                             func=AF.Identity, bias=blin[:, 0:1],
                                             scale=1.0)
                        nc.sync.dma_start(out=FINT[:, r0 + s: r0 + s + w],
                                          in_=fT[:64, :w])
                    else:
                        nc.sync.dma_start(out=XT2[:, r0 + s: r0 + s + w],
                                          in_=xh[:, :w])
                        wbt = wb_u if wb == "u" else wb_i
                        bbt = bb_u if bb == "u" else bb_i
                        y_sb = yp.tile([128, 5, 512], mybir.dt.float32, tag="y")
                        for b in range(5):
                            psy = pp.tile([128, 512], mybir.dt.float32, space="PSUM",
                                          tag=f"py{b % 2}")
                            nc.tensor.matmul(out=psy[:, :w],
                                             lhsT=wbt[:, b * 128:(b + 1) * 128],
                                             rhs=xh[:, :w], start=True, stop=True)
                            nc.scalar.activation(out=y_sb[:, b, :w], in_=psy[:, :w],
                                                 func=AF.Identity, bias=bbt[:, b:b + 1],
                                                 scale=1.0)
                        for b in range(5):
                            nc.sync.dma_start(
                                out=Y[r0 + s: r0 + s + w, b * 128:(b + 1) * 128]
                                .rearrange("r p -> p r"),
                                in_=y_sb[:, b, :w])
    nc.compile()
    return nc


# ---------------------------------------------------------------------------
# Launcher: persistent jits, device-resident flow
# ---------------------------------------------------------------------------

class _Prog:
    def __init__(self, nc):
        import jax
        from concourse import mybir
        from concourse.bass2jax import (_bass_exec_p, partition_id_tensor,
                                        install_neuronx_cc_hook)
        install_neuronx_cc_hook()
        self.nc = nc
        self.partition_name = (nc.partition_id_tensor.name
                               if nc.partition_id_tensor else None)
        self.in_names, self.out_names, self.out_avals = [], [], []
        self._in_avals = {}
        for alloc in nc.m.functions[0].allocations:
            if not isinstance(alloc, mybir.MemoryLocationSet):
                continue
            name = alloc.memorylocations[0].name
            if alloc.kind == "ExternalInput":
                if name != self.partition_name:
                    self.in_names.append(name)
                    self._in_avals[name] = jax.core.ShapedArray(
                        tuple(alloc.tensor_shape), mybir.dt.np(alloc.dtype))
            elif alloc.kind == "ExternalOutput":
                self.out_names.append(name)
                self.out_avals.append(jax.core.ShapedArray(
                    tuple(alloc.tensor_shape), mybir.dt.np(alloc.dtype)))
        self._jit = None

    def _make(self, mesh):
        import jax
        import numpy as np
        from jax.sharding import PartitionSpec
        from jax.experimental.shard_map import shard_map
        from concourse.bass2jax import _bass_exec_p, partition_id_tensor
        n_params = len(self.in_names)
        n_outs = len(self.out_names)
        all_in = list(self.in_names) + list(self.out_names)
        pname = self.partition_name
        if pname is not None:
            all_in.append(pname)
        out_avals = tuple(self.out_avals)
        out_names = tuple(self.out_names)
        nc = self.nc

        def _body(*args):
            operands = list(args)
            if pname is not None:
                operands.append(partition_id_tensor())
            outs = _bass_exec_p.bind(
                *operands,
                out_avals=out_avals,
                in_names=tuple(all_in),
                out_names=out_names,
                lowering_input_output_aliases=(),
                sim_require_finite=False,
                sim_require_nnan=False,
                nc=nc,
            )
            return tuple(outs)

        donate = tuple(range(n_params, n_params + n_outs))
        in_specs = (PartitionSpec("core"),) * (n_params + n_outs)
        out_specs = (PartitionSpec("core"),) * n_outs
        self._jit = jax.jit(
            shard_map(_body, mesh=mesh, in_specs=in_specs, out_specs=out_specs,
                      check_rep=False),
            donate_argnums=donate, keep_unused=True)

    def compile_ahead(self, mesh, shard):
        import jax
        if self._jit is None:
            self._make(mesh)
        specs = []
        for name in self.in_names:
            a = self._in_avals[name]
            specs.append(jax.ShapeDtypeStruct(
                (len(mesh.devices) * a.shape[0],) + tuple(a.shape[1:]), a.dtype,
                sharding=shard))
        for a in self.out_avals:
            specs.append(jax.ShapeDtypeStruct(
                (len(mesh.devices) * a.shape[0],) + tuple(a.shape[1:]), a.dtype,
                sharding=shard))
        self._compiled = self._jit.lower(*specs).compile()

    def __call__(self, mesh, zeros_fn, **kw):
        if self._jit is None:
            self._make(mesh)
        ops = [kw[n] for n in self.in_names]
        zeros = [zeros_fn(tuple(a.shape), a.dtype) for a in self.out_avals]
        fn = getattr(self, "_compiled", None) or self._jit
        outs = fn(*ops, *zeros)
        return dict(zip(self.out_names, outs))


_cache = {}


def kernel(**inp):
    import os, time
    global _LAST_HW_NS, _HW_NS_TOTAL
    t_all = time.time()
    _dbg = os.environ.get("HGT_DEBUG")
    _tp = [time.time()]

    def _t(tag):
        if _dbg:
            now = time.time()
            print(f"[hgt] {tag}: {now - _tp[0]:.3f}s", flush=True)
            _tp[0] = now

    import jax
    import jax.numpy as jnp
    from jax.sharding import Mesh, PartitionSpec, NamedSharding
    from jax.experimental.shard_map import shard_map

    pre = _preprocess(inp)
    W = pre["W"]
    _t("preprocess")

    if "mesh" not in _cache:
        devices = jax.devices()[:NC]
        _cache["mesh"] = Mesh(np.asarray(devices), ("core",))
    mesh = _cache["mesh"]
    shard = NamedSharding(mesh, PartitionSpec("core"))

    tot_u = pre["tab_u"].shape[1]
    tot_i = pre["tab_i"].shape[1]
    key = ("progs", tuple(pre["Ku"]), tuple(pre["Ki"]),
           W["cu0"], W["ci0"], W["cu1"], W["ci1"])
    futs = None
    if key not in _cache:
        from concurrent.futures import ThreadPoolExecutor
        _cache.clear()
        _cache["mesh"] = mesh
        ex = ThreadPoolExecutor(max_workers=6)

        def mk(builder, *a):
            p = _Prog(builder(*a))
            p.compile_ahead(mesh, shard)
            return p

        def mk_glue():
            def _ag(y):
                return jax.lax.all_gather(y, "core", axis=0, tiled=True)

            agj = jax.jit(shard_map(_ag, mesh=mesh,
                                    in_specs=(PartitionSpec("core"),),
                                    out_specs=PartitionSpec("core"),
                                    check_rep=False))
            spec = jax.ShapeDtypeStruct((NC * RALL, 640), np.float32,
                                        sharding=shard)
            agc = agj.lower(spec).compile()
            zjit = jax.jit(jnp.zeros, static_argnums=(0, 1), out_shardings=shard)
            for sh, dt in (((RALL, 640), np.float32), ((128, RALL), np.float32),
                           ((RALL, 128), np.float32), ((64, RALL), np.float32)):
                zjit((NC * sh[0],) + sh[1:], np.dtype(dt))  # warm the compile

            def _zeros(shape, dtype):
                return zjit((NC * shape[0],) + tuple(shape[1:]), dtype)

            return agc, _zeros

        futs = dict(
            p1=ex.submit(mk, _build_P1),
            p2=ex.submit(mk, _build_P2, pre["Ku"], pre["Ki"], tot_u + tot_i),
            p3=ex.submit(mk, _build_P34, 0, False,
                         {"cu": W["cu0"], "ci": W["ci0"]}),
            p4=ex.submit(mk, _build_P34, 1, True,
                         {"cu": W["cu1"], "ci": W["ci1"]}),
            glue=ex.submit(mk_glue),
        )
        _t("threads spawned")

    def rep(a):
        """replicate a per-core tensor: concat 8 copies on axis 0."""
        return np.concatenate([a] * NC, axis=0)

    dev = lambda a: jax.device_put(np.ascontiguousarray(a), shard)

    t0 = time.time()
    args1 = dict(
        xuT=dev(pre["xuT"]), xiT=dev(pre["xiT"]),
        WINU=dev(rep(W["WINU"])), BINU=dev(rep(W["BINU"].reshape(128, 1))),
        WINI=dev(rep(W["WINI"])), BINI=dev(rep(W["BINI"].reshape(128, 1))),
    )
    tab = np.concatenate([
        np.concatenate([pre["tab_u"][c], pre["tab_i"][c]]) for c in range(NC)])
    TAB = dev(tab)
    wb = {}
    for l in range(L):
        wb[f"WBU{l}"] = dev(rep(W[f"WBU{l}"]))
        wb[f"BBU{l}"] = dev(rep(W[f"BBU{l}"]))
        wb[f"WBI{l}"] = dev(rep(W[f"WBI{l}"]))
        wb[f"BBI{l}"] = dev(rep(W[f"BBI{l}"]))
        wb[f"GWOU{l}"] = dev(rep(W[f"GWOU{l}"]))
        wb[f"GBOU{l}"] = dev(rep(W[f"GBOU{l}"].reshape(128, 1)))
        wb[f"GWOI{l}"] = dev(rep(W[f"GWOI{l}"]))
        wb[f"GBOI{l}"] = dev(rep(W[f"GBOI{l}"].reshape(128, 1)))
    wlin = dev(rep(W["WLIN"]))
    blin = dev(rep(W["BLIN"].reshape(128, 1)))
    _t("uploads")
    if futs is not None:
        _cache[key] = (futs["p1"].result(), futs["p2"].result(),
                       futs["p3"].result(), futs["p4"].result(),
                       *futs["glue"].result())
    p1, p2, p3, p4, agj, _zeros = _cache[key]
    _t("program builds+compiles joined")

    r1 = p1(mesh, _zeros, **args1, WBU=wb["WBU0"], BBU=wb["BBU0"],
            WBI=wb["WBI0"], BBI=wb["BBI0"])
    Y, XT = r1["Y"], r1["XT"]
    _t("P1 dispatch")
    Yfull = agj(Y)
    _t("AG1 dispatch")
    OUT = p2(mesh, _zeros, Yfull=Yfull, Yloc=Y, TAB=TAB)["OUT"]
    _t("P2a dispatch")
    r3 = p3(mesh, _zeros, OUT=OUT, XTP=XT,
            GWOU=wb["GWOU0"], GBOU=wb["GBOU0"], GWOI=wb["GWOI0"], GBOI=wb["GBOI0"],
            WBU=wb["WBU1"], BBU=wb["BBU1"], WBI=wb["WBI1"], BBI=wb["BBI1"])
    Y2, XT2 = r3["Y"], r3["XT2"]
    _t("P3 dispatch")
    Yfull2 = agj(Y2)
    OUT2 = p2(mesh, _zeros, Yfull=Yfull2, Yloc=Y2, TAB=TAB)["OUT"]
    _t("AG2+P2b dispatch")
    r4 = p4(mesh, _zeros, OUT=OUT2, XTP=XT2,
            GWOU=wb["GWOU1"], GBOU=wb["GBOU1"], GWOI=wb["GWOI1"], GBOI=wb["GBOI1"],
            WLIN=wlin, BLIN=blin)
    _t("P4 dispatch")
    FINT = np.asarray(r4["FINT"]).reshape(NC, 64, RALL)
    _t("FINT download/block")
    dt_ns = int((time.time() - t0) * 1e9)
    _LAST_HW_NS = dt_ns
    _HW_NS_TOTAL += dt_ns

    out = _assemble(pre, FINT)
    _t("assemble")
    return out


# revision 8
# speedup vs baseline: 4.7068x; 1.0085x over previous
"""HGT kernel for 8 Trainium2 NeuronCores — fully on-device pipeline.

Design (transfer-minimal: the axon tunnel to the devices is ~70MB/s, so the
whole network runs on device; host only preprocesses indices and weights):
  - Nodes are degree-sorted and dealt round-robin across cores (sorted rank i
    -> core i%8, slot i//8), so every core's tile t covers the same degree
    band and one compile-time K_t fits all cores (identical SPMD NEFF).
  - Per core, a Y table [8832, 640] holds (kta|ktb|vta|vtb|q) for its 2560
    user rows + 6272 item rows, with the relation transforms A_k (scaled by
    p_rel/sqrt(D)) and A_v folded into the kqv weights as block-diagonal
    factors on the host.
  - Programs (each its own NEFF, chained through device-resident jax arrays;
    one jit may contain only a single bass_exec, so glue like all_gather and
    zeros-creation runs in separate pure-XLA jits):
      P1: input proj + relu + folded kqv  -> Y, XT          [row-sharded]
      AG: lax.all_gather(Y)               -> Yfull           [XLA jit]
      P2: edge phase (gather, masked segment softmax, weighted sum) -> OUT
          [dst-sharded; gathers k and v rows from Yfull viewed as
           [70656*5, 128] via per-edge indices idx=(row*5+colblock); the v
           gather reuses the same indices with element_offset=2*128]
      P3: gelu -> out linear -> sigmoid-gated skip -> relu -> next kqv
      P4: same head + shared final linear -> FINT [64, 8832]
  - Padding slots point at SENT=12801 (ktb column block of an item row,
    which is identically zero), and are masked to -1e30 before the softmax.
"""

import sys
import numpy as np

sys.path.insert(0, "/opt/trn_rl_repo")

H, D = 8, 16
HID = H * D
NU, NI = 20000, 50000
L = 2
NC = 8
P = 128
RU, RI = 2560, 6272          # per-core padded user/item rows
RALL = RU + RI               # 8832
NFULL = NC * RALL            # 70656
SENT = (RU) * 5 + 1          # 12801: ktb block of core-0 item row 0 (always 0)
INV_SQRT_D = 1.0 / np.sqrt(np.float32(D))

_LAST_HW_NS = None
_HW_NS_TOTAL = 0


# ---------------------------------------------------------------------------
# Host preprocessing
# ---------------------------------------------------------------------------

def _node_assign(deg):
    """degree-sort nodes; sorted rank i -> core i%8, slot i//8.
    Returns (order, core_of, slot_of): order[i] = node id at rank i."""
    order = np.argsort(deg, kind="stable")
    n = deg.shape[0]
    core_of = np.empty(n, np.int32)
    slot_of = np.empty(n, np.int32)
    ranks = np.arange(n)
    core_of[order] = (ranks % NC).astype(np.int32)
    slot_of[order] = (ranks // NC).astype(np.int32)
    return order, core_of, slot_of


def _edge_tables(dst_core, dst_slot, gidx, n_tiles, rows_pc):
    """Build per-core flat slot tables.

    dst_core/dst_slot: per-edge destination (core, slot). gidx: per-edge
    gather index into the [NFULL*5, 128] view. Returns (K_t list, flat int32
    [NC, TOT] table, tile offsets)."""
    E = gidx.shape[0]
    # position of each edge within its destination's list
    key = (dst_core.astype(np.int64) * rows_pc + dst_slot).astype(np.int64)
    order = np.argsort(key, kind="stable")
    ks = key[order]
    grp_start = np.zeros(E, np.int64)
    new_grp = np.ones(E, bool)
    new_grp[1:] = ks[1:] != ks[:-1]
    idx_of_start = np.nonzero(new_grp)[0]
    grp_start[idx_of_start] = idx_of_start
    grp_start = np.maximum.accumulate(np.where(new_grp, np.arange(E), 0))
    pos_sorted = np.arange(E) - grp_start
    pos = np.empty(E, np.int64)
    pos[order] = pos_sorted

    deg_pc = np.zeros((NC, rows_pc), np.int64)
    np.add.at(deg_pc, (dst_core, dst_slot), 1)
    # per-tile K shared across cores
    Kt = []
    for t in range(n_tiles):
        sl = slice(t * P, (t + 1) * P)
        Kt.append(int(deg_pc[:, sl].max()))
    offs = np.zeros(n_tiles + 1, np.int64)
    for t in range(n_tiles):
        offs[t + 1] = offs[t] + P * Kt[t]
    tot = int(offs[-1])
    tab = np.full((NC, tot), SENT, np.int32)
    t_of_slot = dst_slot // P
    p_of_slot = dst_slot % P
    kt_arr = np.asarray(Kt, np.int64)
    flat = offs[t_of_slot] + p_of_slot.astype(np.int64) * kt_arr[t_of_slot] + pos
    tab[dst_core, flat] = gidx.astype(np.int32)
    return Kt, tab, offs


def _blockdiag(blocks):
    out = np.zeros((HID, HID), dtype=np.float32)
    for h in range(H):
        out[h * D:(h + 1) * D, h * D:(h + 1) * D] = blocks[h]
    return out


def _sigmoid(x):
    return 1.0 / (1.0 + np.exp(-np.float64(x)))


def _preprocess(inp):
    """Everything host-side: permutations, edge tables, folded weights."""
    pre = {}
    e_ui = (np.asarray(inp["edge_src_ui"]), np.asarray(inp["edge_dst_ui"]))
    e_iu = (np.asarray(inp["edge_src_iu"]), np.asarray(inp["edge_dst_iu"]))
    e_uu = (np.asarray(inp["edge_src_uu"]), np.asarray(inp["edge_dst_uu"]))

    deg_u = np.bincount(e_iu[1], minlength=NU) + np.bincount(e_uu[1], minlength=NU)
    deg_i = np.bincount(e_ui[1], minlength=NI)
    pre["ord_u"], cu, su = _node_assign(deg_u)
    pre["ord_i"], ci, si = _node_assign(deg_i)

    # flat Y row of each node: user u -> core*RALL + slot ; item -> +RU
    urow = cu.astype(np.int64) * RALL + su
    irow = ci.astype(np.int64) * RALL + RU + si

    # user-dst aggregation: rel iu (item src, kta=col0) + rel uu (user src, ktb=col1)
    dstc = np.concatenate([cu[e_iu[1]], cu[e_uu[1]]])
    dsts = np.concatenate([su[e_iu[1]], su[e_uu[1]]])
    gidx = np.concatenate([irow[e_iu[0]] * 5 + 0, urow[e_uu[0]] * 5 + 1])
    pre["Ku"], pre["tab_u"], _ = _edge_tables(dstc, dsts, gidx, RU // P, RU)

    # item-dst aggregation: rel ui (user src, kta=col0)
    pre["Ki"], pre["tab_i"], _ = _edge_tables(
        ci[e_ui[1]], si[e_ui[1]], urow[e_ui[0]] * 5 + 0, RI // P, RI)

    # permuted x, transposed, per-core concat on axis 0
    x_user = np.asarray(inp["x_user"], np.float32)
    x_item = np.asarray(inp["x_item"], np.float32)
    xuT = np.zeros((NC, 128, RU), np.float32)
    xiT = np.zeros((NC, 64, RI), np.float32)
    xuT[cu, :, su] = x_user            # fancy index: rows to (core, :, slot)
    xiT[ci, :, si] = x_item
    pre["xuT"] = xuT.reshape(NC * 128, RU)
    pre["xiT"] = xiT.reshape(NC * 64, RI)

    # ---- fold weights ----
    A_k = np.asarray(inp["A_k"], np.float32)
    A_v = np.asarray(inp["A_v"], np.float32)
    p_rel = np.asarray(inp["p_rel"], np.float32)
    W = {}
    for l in range(L):
        Wk_u, Wq_u, Wv_u = np.split(np.asarray(inp["W_kqv_user"][l], np.float32), 3, axis=1)
        bk_u, bq_u, bv_u = np.split(np.asarray(inp["b_kqv_user"][l], np.float32), 3)
        Wk_i, Wq_i, Wv_i = np.split(np.asarray(inp["W_kqv_item"][l], np.float32), 3, axis=1)
        bk_i, bq_i, bv_i = np.split(np.asarray(inp["b_kqv_item"][l], np.float32), 3)

        def bk(r):
            return _blockdiag(A_k[l, r] * (p_rel[l, r] * INV_SQRT_D)[:, None, None])

        Bk0, Bk1, Bk2 = bk(0), bk(1), bk(2)
        Bv0, Bv1, Bv2 = (_blockdiag(A_v[l, r]) for r in range(3))
        # user cols: kta=k@Bk0, ktb=k@Bk2, vta=v@Bv0, vtb=v@Bv2, q
        W[f"WBU{l}"] = np.concatenate(
            [Wk_u @ Bk0, Wk_u @ Bk2, Wv_u @ Bv0, Wv_u @ Bv2, Wq_u], axis=1)
        W[f"BBU{l}"] = np.stack(
            [bk_u @ Bk0, bk_u @ Bk2, bv_u @ Bv0, bv_u @ Bv2, bq_u], axis=1)  # [128,5]
        # item cols: kta=k@Bk1, ktb=0, vta=v@Bv1, vtb=0, q
        Z = np.zeros((HID, HID), np.float32)
        W[f"WBI{l}"] = np.concatenate([Wk_i @ Bk1, Z, Wv_i @ Bv1, Z, Wq_i], axis=1)
        W[f"BBI{l}"] = np.stack(
            [bk_i @ Bk1, np.zeros(HID, np.float32), bv_i @ Bv1,
             np.zeros(HID, np.float32), bq_i], axis=1)
        g_u = np.float32(_sigmoid(inp["skip_user"][l]))
        g_i = np.float32(_sigmoid(inp["skip_item"][l]))
        W[f"GWOU{l}"] = g_u * np.asarray(inp["W_out_user"][l], np.float32)
        W[f"GBOU{l}"] = g_u * np.asarray(inp["b_out_user"][l], np.float32)
        W[f"GWOI{l}"] = g_i * np.asarray(inp["W_out_item"][l], np.float32)
        W[f"GBOI{l}"] = g_i * np.asarray(inp["b_out_item"][l], np.float32)
        W[f"cu{l}"] = float(1.0 - g_u)
        W[f"ci{l}"] = float(1.0 - g_i)
    Wlin = np.zeros((128, 128), np.float32)
    Wlin[:, :64] = np.asarray(inp["W_lin"], np.float32)
    W["WLIN"] = Wlin
    blin = np.zeros(128, np.float32)
    blin[:64] = np.asarray(inp["b_lin"], np.float32)
    W["BLIN"] = blin
    W["WINU"] = np.asarray(inp["W_in_user"], np.float32)
    W["BINU"] = np.asarray(inp["b_in_user"], np.float32)
    W["WINI"] = np.asarray(inp["W_in_item"], np.float32)
    W["BINI"] = np.asarray(inp["b_in_item"], np.float32)
    pre["W"] = W
    return pre


# ---------------------------------------------------------------------------
# Numpy emulation of the device pipeline (for validation / fallback)
# ---------------------------------------------------------------------------

def _np_edge_phase(Yfull, Ylocal_all, Klist, tab, row0):
    """Per-core edge phase, all cores at once. Returns OUT rows [NC, ntiles*128, 128]."""
    Yv = Yfull.reshape(-1, 128)      # [NFULL*5, 128]
    ntiles = len(Klist)
    out = np.zeros((NC, ntiles * P, HID), np.float32)
    for c in range(NC):
        off = 0
        for t in range(ntiles):
            K = Klist[t]
            if K == 0:
                continue
            idx = tab[c, off:off + P * K].reshape(P, K)
            off += P * K
            kg = Yv[idx]                       # [128, K, 128]
            vg = Yv[idx + 2]                   # element_offset 2 blocks
            q = Ylocal_all[c, row0 + t * P: row0 + (t + 1) * P, 512:640]
            s = (kg.reshape(P, K, H, D) * q.reshape(P, 1, H, D)).sum(-1)  # [128,K,H]
            s = s - 1e30 * (idx == SENT)[:, :, None]
            m = s.max(axis=1, keepdims=True)
            e = np.exp(s - m)
            den = e.sum(axis=1, keepdims=True)
            alpha = e / den
            o = (vg.reshape(P, K, H, D) * alpha[..., None]).sum(axis=1)
            out[c, t * P:(t + 1) * P] = o.reshape(P, HID)
    return out


def _np_pipeline(pre):
    """Numpy emulation of P1->P2->P3->P2->P4. Returns FINT [NC, 64, RALL]."""
    W = pre["W"]
    xuT = pre["xuT"].reshape(NC, 128, RU)
    xiT = pre["xiT"].reshape(NC, 64, RI)
    from scipy.special import erf

    def gelu(x):
        return 0.5 * x * (1.0 + erf(x / np.sqrt(2.0))).astype(np.float32)

    XT = np.zeros((NC, 128, RALL), np.float32)
    for c in range(NC):
        XT[c, :, :RU] = np.maximum(W["WINU"].T @ xuT[c] + W["BINU"][:, None], 0.0)
        XT[c, :, RU:] = np.maximum(W["WINI"].T @ xiT[c] + W["BINI"][:, None], 0.0)

    for l in range(L):
        Y = np.zeros((NC, RALL, 640), np.float32)
        # bias layout: BB [128, 5] -> col block b gets BB[:, b]
        for c in range(NC):
            bb_u = np.concatenate([W[f"BBU{l}"][:, b] for b in range(5)])
            bb_i = np.concatenate([W[f"BBI{l}"][:, b] for b in range(5)])
            Y[c, :RU] = XT[c, :, :RU].T @ W[f"WBU{l}"] + bb_u[None, :]
            Y[c, RU:] = XT[c, :, RU:].T @ W[f"WBI{l}"] + bb_i[None, :]
        Yfull = Y.reshape(NFULL, 640)
        out_u = _np_edge_phase(Yfull, Y, pre["Ku"], pre["tab_u"], 0)
        out_i = _np_edge_phase(Yfull, Y, pre["Ki"], pre["tab_i"], RU)
        XT2 = np.zeros_like(XT)
        for c in range(NC):
            aTu = W[f"GWOU{l}"].T @ gelu(out_u[c]).T + W[f"GBOU{l}"][:, None]
            aTi = W[f"GWOI{l}"].T @ gelu(out_i[c]).T + W[f"GBOI{l}"][:, None]
            XT2[c, :, :RU] = np.maximum(aTu + W[f"cu{l}"] * XT[c, :, :RU], 0.0)
            XT2[c, :, RU:] = np.maximum(aTi + W[f"ci{l}"] * XT[c, :, RU:], 0.0)
        XT = XT2
    FINT = np.zeros((NC, 64, RALL), np.float32)
    for c in range(NC):
        FINT[c] = (W["WLIN"].T @ XT[c] + W["BLIN"][:, None])[:64]
    return FINT


def _assemble(pre, FINT):
    """FINT [NC, 64, RALL] -> full output [NU+NI, 64] in original order."""
    out = np.zeros((NU + NI, 64), np.float32)
    un = FINT[:, :, :NU // NC]            # [NC, 64, 2500]
    A = un.transpose(2, 0, 1).reshape(NU, 64)   # rank i = 8*slot + core
    out[pre["ord_u"]] = A
    it = FINT[:, :, RU:RU + NI // NC]
    Bm = it.transpose(2, 0, 1).reshape(NI, 64)
    out[NU + np.asarray(pre["ord_i"])] = Bm
    return out


def kernel_numpy(**inp):
    pre = _preprocess(inp)
    return _assemble(pre, _np_pipeline(pre))


# ---------------------------------------------------------------------------
# Bass programs
# ---------------------------------------------------------------------------

def _slabs(n, w=512):
    out = []
    o = 0
    while o < n:
        out.append((o, min(w, n - o)))
        o += min(w, n - o)
    return out


def _build_P1():
    import concourse.bacc as bacc
    import concourse.mybir as mybir
    import concourse.tile as tile

    nc = bacc.Bacc("TRN2", target_bir_lowering=False, debug=False)
    xuT = nc.dram_tensor("xuT", [128, RU], mybir.dt.float32, kind="ExternalInput")
    xiT = nc.dram_tensor("xiT", [64, RI], mybir.dt.float32, kind="ExternalInput")
    WINU = nc.dram_tensor("WINU", [128, 128], mybir.dt.float32, kind="ExternalInput")
    BINU = nc.dram_tensor("BINU", [128, 1], mybir.dt.float32, kind="ExternalInput")
    WINI = nc.dram_tensor("WINI", [64, 128], mybir.dt.float32, kind="ExternalInput")
    BINI = nc.dram_tensor("BINI", [128, 1], mybir.dt.float32, kind="ExternalInput")
    WBU = nc.dram_tensor("WBU", [128, 640], mybir.dt.float32, kind="ExternalInput")
    BBU = nc.dram_tensor("BBU", [128, 5], mybir.dt.float32, kind="ExternalInput")
    WBI = nc.dram_tensor("WBI", [128, 640], mybir.dt.float32, kind="ExternalInput")
    BBI = nc.dram_tensor("BBI", [128, 5], mybir.dt.float32, kind="ExternalInput")
    Y = nc.dram_tensor("Y", [RALL, 640], mybir.dt.float32, kind="ExternalOutput")
    XT = nc.dram_tensor("XT", [128, RALL], mybir.dt.float32, kind="ExternalOutput")

    AF = mybir.ActivationFunctionType
    with tile.TileContext(nc) as tc:
        with (
            tc.tile_pool(name="w", bufs=1) as wp,
            tc.tile_pool(name="x", bufs=3) as xp,
            tc.tile_pool(name="y", bufs=3) as yp,
            tc.tile_pool(name="ps", bufs=2, space="PSUM") as pp,
        ):
            w_in_u = wp.tile([128, 128], mybir.dt.float32, tag="wiu")
            nc.sync.dma_start(out=w_in_u[:], in_=WINU[:, :])
            w_in_i = wp.tile([64, 128], mybir.dt.float32, tag="wii")
            nc.sync.dma_start(out=w_in_i[:], in_=WINI[:, :])
            b_in_u = wp.tile([128, 1], mybir.dt.float32, tag="biu")
            nc.sync.dma_start(out=b_in_u[:], in_=BINU[:, :])
            b_in_i = wp.tile([128, 1], mybir.dt.float32, tag="bii")
            nc.sync.dma_start(out=b_in_i[:], in_=BINI[:, :])
            wb_u = wp.tile([128, 640], mybir.dt.float32, tag="wbu")
            nc.sync.dma_start(out=wb_u[:], in_=WBU[:, :])
            wb_i = wp.tile([128, 640], mybir.dt.float32, tag="wbi")
            nc.sync.dma_start(out=wb_i[:], in_=WBI[:, :])
            bb_u = wp.tile([128, 5], mybir.dt.float32, tag="bbu")
            nc.sync.dma_start(out=bb_u[:], in_=BBU[:, :])
            bb_i = wp.tile([128, 5], mybir.dt.float32, tag="bbi")
            nc.sync.dma_start(out=bb_i[:], in_=BBI[:, :])

            for (xT_d, w_in, b_in, wb, bb, kin, r0, rows) in (
                (xuT, w_in_u, b_in_u, wb_u, bb_u, 128, 0, RU),
                (xiT, w_in_i, b_in_i, wb_i, bb_i, 64, RU, RI),
            ):
                for (s, w) in _slabs(rows):
                    x_sb = xp.tile([kin, 512], mybir.dt.float32, tag="x")
                    nc.sync.dma_start(out=x_sb[:, :w], in_=xT_d[:, s:s + w])
                    psx = pp.tile([128, 512], mybir.dt.float32, space="PSUM", tag="px")
                    nc.tensor.matmul(out=psx[:, :w], lhsT=w_in[:, :], rhs=x_sb[:, :w],
                                     start=True, stop=True)
                    xh = xp.tile([128, 512], mybir.dt.float32, tag="xh")
                    nc.scalar.activation(out=xh[:, :w], in_=psx[:, :w], func=AF.Relu,
                                         bias=b_in[:, 0:1], scale=1.0)
                    nc.sync.dma_start(out=XT[:, r0 + s: r0 + s + w], in_=xh[:, :w])
                    y_sb = yp.tile([128, 5, 512], mybir.dt.float32, tag="y")
                    for b in range(5):
                        psy = pp.tile([128, 512], mybir.dt.float32, space="PSUM",
                                      tag=f"py{b % 2}")
                        nc.tensor.matmul(out=psy[:, :w], lhsT=wb[:, b * 128:(b + 1) * 128],
                                         rhs=xh[:, :w], start=True, stop=True)
                        nc.scalar.activation(out=y_sb[:, b, :w], in_=psy[:, :w],
                                             func=AF.Identity, bias=bb[:, b:b + 1],
                                             scale=1.0)
                    for b in range(5):
                        nc.sync.dma_start(
                            out=Y[r0 + s: r0 + s + w, b * 128:(b + 1) * 128]
                            .rearrange("r p -> p r"),
                            in_=y_sb[:, b, :w])
    nc.compile()
    return nc


def _build_P2(Klist_u, Klist_i, tot):
    import concourse.bacc as bacc
    import concourse.mybir as mybir
    import concourse.tile as tile
    from concourse import bass

    nc = bacc.Bacc("TRN2", target_bir_lowering=False, debug=False)
    Yfull = nc.dram_tensor("Yfull", [NFULL, 640], mybir.dt.float32, kind="ExternalInput")
    Yloc = nc.dram_tensor("Yloc", [RALL, 640], mybir.dt.float32, kind="ExternalInput")
    TAB = nc.dram_tensor("TAB", [tot], mybir.dt.int32, kind="ExternalInput")
    OUT = nc.dram_tensor("OUT", [RALL, 128], mybir.dt.float32, kind="ExternalOutput")

    AF = mybir.ActivationFunctionType
    ALU = mybir.AluOpType
    Yv = Yfull[:, :].rearrange("a (b c) -> (a b) c", b=5, c=128)

    with tile.TileContext(nc) as tc:
        with (
            tc.tile_pool(name="ix", bufs=2) as ixp,
            tc.tile_pool(name="kv", bufs=2) as kvp,
            tc.tile_pool(name="sm", bufs=2) as smp,
            tc.tile_pool(name="o", bufs=2) as op,
        ):
            off = 0
            tlist = [(t, K, t * P) for t, K in enumerate(Klist_u)] + \
                    [(len(Klist_u) + t, K, RU + t * P) for t, K in enumerate(Klist_i)]
            for (tt, K, row0) in tlist:
                if K == 0:
                    oz = op.tile([128, 128], mybir.dt.float32, tag="oz")
                    nc.vector.memset(oz[:], 0.0)
                    nc.sync.dma_start(out=OUT[row0:row0 + P, :], in_=oz[:])
                    continue
                idx = ixp.tile([128, K], mybir.dt.int32, tag="idx")
                nc.sync.dma_start(
                    out=idx[:],
                    in_=TAB[off:off + P * K].rearrange("(p k) -> p k", p=128, k=K))
                off += P * K
                # one gather per slot: 3 contiguous 128-blocks from row idx
                # (k at block 0, v at block +2; middle block unused)
                gath = kvp.tile([128, K * 384], mybir.dt.float32, tag="kg")
                for k in range(K):
                    nc.gpsimd.indirect_dma_start(
                        out=gath[:, k * 384:(k + 1) * 384], out_offset=None, in_=Yv,
                        in_offset=bass.IndirectOffsetOnAxis(ap=idx[:, k:k + 1], axis=0))
                g3 = gath[:].rearrange("p (k c) -> p k c", k=K, c=384)
                kg4 = g3[:, :, 0:128].rearrange("p k (h d) -> p k h d", h=H, d=D)
                vg4 = g3[:, :, 256:384].rearrange("p k (h d) -> p k h d", h=H, d=D)
                q = smp.tile([128, 128], mybir.dt.float32, tag="q")
                nc.sync.dma_start(out=q[:], in_=Yloc[row0:row0 + P, 512:640].rearrange(
                    "r c -> r c"))
                # qk product in-place into kg
                qb = q[:].rearrange("p (h d) -> p h d", h=H, d=D) \
                    .unsqueeze(1).broadcast_to([128, K, H, D])
                nc.vector.tensor_tensor(out=kg4, in0=kg4, in1=qb, op=ALU.mult)
                # scores [128, K, H]
                s = smp.tile([128, K * H], mybir.dt.float32, tag="s")
                s3 = s[:].rearrange("p (k h) -> p k h", k=K, h=H)
                nc.vector.tensor_reduce(out=s3, in_=kg4, axis=mybir.AxisListType.X,
                                        op=ALU.add)
                # mask pad slots
                mskf = smp.tile([128, K], mybir.dt.float32, tag="mf")
                nc.vector.tensor_copy(out=mskf[:], in_=idx[:])
                nc.vector.tensor_scalar(out=mskf[:], in0=mskf[:],
                                        scalar1=float(SENT), scalar2=1e30,
                                        op0=ALU.is_equal, op1=ALU.mult)
                mb = mskf[:].unsqueeze(2).broadcast_to([128, K, H])
                nc.vector.tensor_tensor(out=s3, in0=s3, in1=mb, op=ALU.subtract)
                # softmax over K per head
                m = smp.tile([128, H], mybir.dt.float32, tag="m")
                sT = s3.transpose([0, 2, 1])       # [p, h, k]
                nc.vector.tensor_reduce(out=m[:], in_=sT, axis=mybir.AxisListType.X,
                                        op=ALU.max)
                mb2 = m[:].unsqueeze(1).broadcast_to([128, K, H])
                nc.vector.tensor_tensor(out=s3, in0=s3, in1=mb2, op=ALU.subtract)
                nc.scalar.activation(out=s[:], in_=s[:], func=AF.Exp)
                den = smp.tile([128, H], mybir.dt.float32, tag="den")
                nc.vector.tensor_reduce(out=den[:], in_=sT, axis=mybir.AxisListType.X,
                                        op=ALU.add)
                rden = smp.tile([128, H], mybir.dt.float32, tag="rden")
                nc.vector.reciprocal(out=rden[:], in_=den[:])
                rb = rden[:].unsqueeze(1).broadcast_to([128, K, H])
                nc.vector.tensor_tensor(out=s3, in0=s3, in1=rb, op=ALU.mult)
                # weighted values: vg *= alpha ; reduce over K
                ab = s3.unsqueeze(3).broadcast_to([128, K, H, D])
                nc.vector.tensor_tensor(out=vg4, in0=vg4, in1=ab, op=ALU.mult)
                o = op.tile([128, 128], mybir.dt.float32, tag="o")
                o3 = o[:].rearrange("p (h d) -> p h d", h=H, d=D)
                vgr = vg4.transpose([0, 2, 3, 1])   # [p, h, d, k]
                nc.vector.tensor_reduce(out=o3, in_=vgr, axis=mybir.AxisListType.X,
                                        op=ALU.add)
                nc.sync.dma_start(out=OUT[row0:row0 + P, :], in_=o[:])
    nc.compile()
    return nc


def _build_P34(lidx, final, consts):
    """P3 (next-layer kqv) or P4 (final linear). consts: dict with folded weights
    as compile-time values for (1-g); weights come in as tensors."""
    import concourse.bacc as bacc
    import concourse.mybir as mybir
    import concourse.tile as tile
    from concourse.masks import make_identity

    nc = bacc.Bacc("TRN2", target_bir_lowering=False, debug=False)
    OUT = nc.dram_tensor("OUT", [RALL, 128], mybir.dt.float32, kind="ExternalInput")
    XTP = nc.dram_tensor("XTP", [128, RALL], mybir.dt.float32, kind="ExternalInput")
    GWOU = nc.dram_tensor("GWOU", [128, 128], mybir.dt.float32, kind="ExternalInput")
    GBOU = nc.dram_tensor("GBOU", [128, 1], mybir.dt.float32, kind="ExternalInput")
    GWOI = nc.dram_tensor("GWOI", [128, 128], mybir.dt.float32, kind="ExternalInput")
    GBOI = nc.dram_tensor("GBOI", [128, 1], mybir.dt.float32, kind="ExternalInput")
    if final:
        WLIN = nc.dram_tensor("WLIN", [128, 128], mybir.dt.float32, kind="ExternalInput")
        BLIN = nc.dram_tensor("BLIN", [128, 1], mybir.dt.float32, kind="ExternalInput")
        FINT = nc.dram_tensor("FINT", [64, RALL], mybir.dt.float32, kind="ExternalOutput")
    else:
        WBU = nc.dram_tensor("WBU", [128, 640], mybir.dt.float32, kind="ExternalInput")
        BBU = nc.dram_tensor("BBU", [128, 5], mybir.dt.float32, kind="ExternalInput")
        WBI = nc.dram_tensor("WBI", [128, 640], mybir.dt.float32, kind="ExternalInput")
        BBI = nc.dram_tensor("BBI", [128, 5], mybir.dt.float32, kind="ExternalInput")
        Y = nc.dram_tensor("Y", [RALL, 640], mybir.dt.float32, kind="ExternalOutput")
        XT2 = nc.dram_tensor("XT2", [128, RALL], mybir.dt.float32, kind="ExternalOutput")

    cu, ci = consts["cu"], consts["ci"]
    AF = mybir.ActivationFunctionType
    ALU = mybir.AluOpType
    with tile.TileContext(nc) as tc:
        with (
            tc.tile_pool(name="w", bufs=1) as wp,
            tc.tile_pool(name="x", bufs=3) as xp,
            tc.tile_pool(name="y", bufs=3) as yp,
            tc.tile_pool(name="ps", bufs=2, space="PSUM") as pp,
        ):
            ident = wp.tile([128, 128], mybir.dt.float32, tag="id")
            make_identity(nc, ident[:])
            gwo_u = wp.tile([128, 128], mybir.dt.float32, tag="gwu")
            nc.sync.dma_start(out=gwo_u[:], in_=GWOU[:, :])
            gwo_i = wp.tile([128, 128], mybir.dt.float32, tag="gwi")
            nc.sync.dma_start(out=gwo_i[:], in_=GWOI[:, :])
            gbo_u = wp.tile([128, 1], mybir.dt.float32, tag="gbu")
            nc.sync.dma_start(out=gbo_u[:], in_=GBOU[:, :])
            gbo_i = wp.tile([128, 1], mybir.dt.float32, tag="gbi")
            nc.sync.dma_start(out=gbo_i[:], in_=GBOI[:, :])
            if final:
                wlin = wp.tile([128, 128], mybir.dt.float32, tag="wl")
                nc.sync.dma_start(out=wlin[:], in_=WLIN[:, :])
                blin = wp.tile([128, 1], mybir.dt.float32, tag="bl")
                nc.sync.dma_start(out=blin[:], in_=BLIN[:, :])
            else:
                wb_u = wp.tile([128, 640], mybir.dt.float32, tag="wbu")
                nc.sync.dma_start(out=wb_u[:], in_=WBU[:, :])
                wb_i = wp.tile([128, 640], mybir.dt.float32, tag="wbi")
                nc.sync.dma_start(out=wb_i[:], in_=WBI[:, :])
                bb_u = wp.tile([128, 5], mybir.dt.float32, tag="bbu")
                nc.sync.dma_start(out=bb_u[:], in_=BBU[:, :])
                bb_i = wp.tile([128, 5], mybir.dt.float32, tag="bbi")
                nc.sync.dma_start(out=bb_i[:], in_=BBI[:, :])

            for (gwo, gbo, cc, wb, bb, r0, rows) in (
                (gwo_u, gbo_u, cu, "u", "u", 0, RU),
                (gwo_i, gbo_i, ci, "i", "i", RU, RI),
            ):
                for (s, w) in _slabs(rows):
                    gt = xp.tile([128, 512], mybir.dt.float32, tag="gt")
                    nsub = (w + 127) // 128
                    for sub in range(nsub):
                        ww = min(128, w - sub * 128)
                        ot = xp.tile([128, 128], mybir.dt.float32, tag="ot")
                        nc.sync.dma_start(
                            out=ot[:ww, :],
                            in_=OUT[r0 + s + sub * 128: r0 + s + sub * 128 + ww, :])
                        pst = pp.tile([128, 128], mybir.dt.float32, space="PSUM",
                                      tag="pt")
                        nc.tensor.transpose(out=pst[:, :ww], in_=ot[:ww, :],
                                            identity=ident[:])
                        # gelu(x) ~ 0.5 x (1 + tanh(0.79788456 x + 0.03567741 x^3))
                        gu = xp.tile([128, 128], mybir.dt.float32, tag="gu")
                        nc.scalar.square(out=gu[:, :ww], in_=pst[:, :ww])
                        nc.vector.tensor_scalar(
                            out=gu[:, :ww], in0=gu[:, :ww], scalar1=0.035677408,
                            scalar2=0.7978845608, op0=ALU.mult, op1=ALU.add)
                        nc.vector.tensor_tensor(out=gu[:, :ww], in0=gu[:, :ww],
                                                in1=pst[:, :ww], op=ALU.mult)
                        nc.scalar.activation(out=gu[:, :ww], in_=gu[:, :ww],
                                             func=AF.Tanh)
                        nc.vector.tensor_scalar(
                            out=gu[:, :ww], in0=gu[:, :ww], scalar1=1.0,
                            scalar2=0.5, op0=ALU.add, op1=ALU.mult)
                        nc.vector.tensor_tensor(
                            out=gt[:, sub * 128: sub * 128 + ww], in0=gu[:, :ww],
                            in1=pst[:, :ww], op=ALU.mult)
                    psa = pp.tile([128, 512], mybir.dt.float32, space="PSUM", tag="pa")
                    nc.tensor.matmul(out=psa[:, :w], lhsT=gwo[:, :], rhs=gt[:, :w],
                                     start=True, stop=True)
                    aT = xp.tile([128, 512], mybir.dt.float32, tag="aT")
                    nc.scalar.activation(out=aT[:, :w], in_=psa[:, :w],
                                         func=AF.Identity, bias=gbo[:, 0:1], scale=1.0)
                    xprev = xp.tile([128, 512], mybir.dt.float32, tag="xp")
                    nc.sync.dma_start(out=xprev[:, :w], in_=XTP[:, r0 + s: r0 + s + w])
                    nc.vector.tensor_scalar_mul(out=xprev[:, :w],
                                                in0=xprev[:, :w], scalar1=cc)
                    nc.vector.tensor_tensor(out=aT[:, :w], in0=aT[:, :w],
                                            in1=xprev[:, :w], op=ALU.add)
                    xh = xp.tile([128, 512], mybir.dt.float32, tag="xh")
                    nc.scalar.activation(out=xh[:, :w], in_=aT[:, :w], func=AF.Relu)
                    if final:
                        psf = pp.tile([128, 512], mybir.dt.float32, space="PSUM",
                                      tag="pf")
                        nc.tensor.matmul(out=psf[:, :w], lhsT=wlin[:, :], rhs=xh[:, :w],
                                         start=True, stop=True)
                        fT = yp.tile([128, 512], mybir.dt.float32, tag="fT")
                        nc.scalar.activation(out=fT[:, :w], in_=psf[:, :w],
                                             func=AF.Identity, bias=blin[:, 0:1],
                                             scale=1.0)
                        nc.sync.dma_start(out=FINT[:, r0 + s: r0 + s + w],
                                          in_=fT[:64, :w])
                    else:
                        nc.sync.dma_start(out=XT2[:, r0 + s: r0 + s + w],
                                          in_=xh[:, :w])
                        wbt = wb_u if wb == "u" else wb_i
                        bbt = bb_u if bb == "u" else bb_i
                        y_sb = yp.tile([128, 5, 512], mybir.dt.float32, tag="y")
                        for b in range(5):
                            psy = pp.tile([128, 512], mybir.dt.float32, space="PSUM",
                                          tag=f"py{b % 2}")
                            nc.tensor.matmul(out=psy[:, :w],
                                             lhsT=wbt[:, b * 128:(b + 1) * 128],
                                             rhs=xh[:, :w], start=True, stop=True)
                            nc.scalar.activation(out=y_sb[:, b, :w], in_=psy[:, :w],
                                                 func=AF.Identity, bias=bbt[:, b:b + 1],
                                                 scale=1.0)
                        for b in range(5):
                            nc.sync.dma_start(
                                out=Y[r0 + s: r0 + s + w, b * 128:(b + 1) * 128]
                                .rearrange("r p -> p r"),
                                in_=y_sb[:, b, :w])
    nc.compile()
    return nc


# ---------------------------------------------------------------------------
# Launcher: persistent jits, device-resident flow
# ---------------------------------------------------------------------------

class _Prog:
    def __init__(self, nc):
        import jax
        from concourse import mybir
        from concourse.bass2jax import (_bass_exec_p, partition_id_tensor,
                                        install_neuronx_cc_hook)
        install_neuronx_cc_hook()
        self.nc = nc
        self.partition_name = (nc.partition_id_tensor.name
                               if nc.partition_id_tensor else None)
        self.in_names, self.out_names, self.out_avals = [], [], []
        self._in_avals = {}
        for alloc in nc.m.functions[0].allocations:
            if not isinstance(alloc, mybir.MemoryLocationSet):
                continue
            name = alloc.memorylocations[0].name
            if alloc.kind == "ExternalInput":
                if name != self.partition_name:
                    self.in_names.append(name)
                    self._in_avals[name] = jax.core.ShapedArray(
                        tuple(alloc.tensor_shape), mybir.dt.np(alloc.dtype))
            elif alloc.kind == "ExternalOutput":
                self.out_names.append(name)
                self.out_avals.append(jax.core.ShapedArray(
                    tuple(alloc.tensor_shape), mybir.dt.np(alloc.dtype)))
        self._jit = None

    def _make(self, mesh):
        import jax
        import numpy as np
        from jax.sharding import PartitionSpec
        from jax.experimental.shard_map import shard_map
        from concourse.bass2jax import _bass_exec_p, partition_id_tensor
        n_params = len(self.in_names)
        n_outs = len(self.out_names)
        all_in = list(self.in_names) + list(self.out_names)
        pname = self.partition_name
        if pname is not None:
            all_in.append(pname)
        out_avals = tuple(self.out_avals)
        out_names = tuple(self.out_names)
        nc = self.nc

        def _body(*args):
            operands = list(args)
            if pname is not None:
                operands.append(partition_id_tensor())
            outs = _bass_exec_p.bind(
                *operands,
                out_avals=out_avals,
                in_names=tuple(all_in),
                out_names=out_names,
                lowering_input_output_aliases=(),
                sim_require_finite=False,
                sim_require_nnan=False,
                nc=nc,
            )
            return tuple(outs)

        donate = tuple(range(n_params, n_params + n_outs))
        in_specs = (PartitionSpec("core"),) * (n_params + n_outs)
        out_specs = (PartitionSpec("core"),) * n_outs
        self._jit = jax.jit(
            shard_map(_body, mesh=mesh, in_specs=in_specs, out_specs=out_specs,
                      check_rep=False),
            donate_argnums=donate, keep_unused=True)

    def compile_ahead(self, mesh, shard):
        import jax
        if self._jit is None:
            self._make(mesh)
        specs = []
        for name in self.in_names:
            a = self._in_avals[name]
            specs.append(jax.ShapeDtypeStruct(
                (len(mesh.devices) * a.shape[0],) + tuple(a.shape[1:]), a.dtype,
                sharding=shard))
        for a in self.out_avals:
            specs.append(jax.ShapeDtypeStruct(
                (len(mesh.devices) * a.shape[0],) + tuple(a.shape[1:]), a.dtype,
                sharding=shard))
        self._compiled = self._jit.lower(*specs).compile()

    def __call__(self, mesh, zeros_fn, **kw):
        if self._jit is None:
            self._make(mesh)
        ops = [kw[n] for n in self.in_names]
        zeros = [zeros_fn(tuple(a.shape), a.dtype) for a in self.out_avals]
        outs = self._jit(*ops, *zeros)
        return dict(zip(self.out_names, outs))


_cache = {}


def kernel(**inp):
    import os, time
    global _LAST_HW_NS, _HW_NS_TOTAL
    t_all = time.time()
    _dbg = os.environ.get("HGT_DEBUG")
    _tp = [time.time()]

    def _t(tag):
        if _dbg:
            now = time.time()
            print(f"[hgt] {tag}: {now - _tp[0]:.3f}s", flush=True)
            _tp[0] = now

    import jax
    import jax.numpy as jnp
    from jax.sharding import Mesh, PartitionSpec, NamedSharding
    from jax.experimental.shard_map import shard_map

    pre = _preprocess(inp)
    W = pre["W"]
    _t("preprocess")

    if "mesh" not in _cache:
        devices = jax.devices()[:NC]
        _cache["mesh"] = Mesh(np.asarray(devices), ("core",))
    mesh = _cache["mesh"]
    shard = NamedSharding(mesh, PartitionSpec("core"))

    tot_u = pre["tab_u"].shape[1]
    tot_i = pre["tab_i"].shape[1]
    key = ("progs", tuple(pre["Ku"]), tuple(pre["Ki"]),
           W["cu0"], W["ci0"], W["cu1"], W["ci1"])
    futs = None
    if key not in _cache:
        from concurrent.futures import ThreadPoolExecutor
        _cache.clear()
        _cache["mesh"] = mesh
        ex = ThreadPoolExecutor(max_workers=6)

        def mk(builder, *a):
            return _Prog(builder(*a))

        def mk_glue():
            def _ag(y):
                return jax.lax.all_gather(y, "core", axis=0, tiled=True)

            agj = jax.jit(shard_map(_ag, mesh=mesh,
                                    in_specs=(PartitionSpec("core"),),
                                    out_specs=PartitionSpec("core"),
                                    check_rep=False))
            zjit = jax.jit(jnp.zeros, static_argnums=(0, 1), out_shardings=shard)
            for sh, dt in (((RALL, 640), np.float32), ((128, RALL), np.float32),
                           ((RALL, 128), np.float32), ((64, RALL), np.float32)):
                zjit((NC * sh[0],) + sh[1:], np.dtype(dt))  # warm the compile

            def _zeros(shape, dtype):
                return zjit((NC * shape[0],) + tuple(shape[1:]), dtype)

            return agj, _zeros

        futs = dict(
            p1=ex.submit(mk, _build_P1),
            p2=ex.submit(mk, _build_P2, pre["Ku"], pre["Ki"], tot_u + tot_i),
            p3=ex.submit(mk, _build_P34, 0, False,
                         {"cu": W["cu0"], "ci": W["ci0"]}),
            p4=ex.submit(mk, _build_P34, 1, True,
                         {"cu": W["cu1"], "ci": W["ci1"]}),
            glue=ex.submit(mk_glue),
        )
        _t("threads spawned")

    def rep(a):
        """replicate a per-core tensor: concat 8 copies on axis 0."""
        return np.concatenate([a] * NC, axis=0)

    dev = lambda a: jax.device_put(np.ascontiguousarray(a), shard)

    t0 = time.time()
    tab = np.concatenate([
        np.concatenate([pre["tab_u"][c], pre["tab_i"][c]]) for c in range(NC)])
    names = ["xuT", "xiT", "WINU", "BINU", "WINI", "BINI", "TAB"]
    arrs = [pre["xuT"], pre["xiT"], rep(W["WINU"]), rep(W["BINU"].reshape(128, 1)),
            rep(W["WINI"]), rep(W["BINI"].reshape(128, 1)), tab]
    for l in range(L):
        for nm, a in ((f"WBU{l}", W[f"WBU{l}"]), (f"BBU{l}", W[f"BBU{l}"]),
                      (f"WBI{l}", W[f"WBI{l}"]), (f"BBI{l}", W[f"BBI{l}"]),
                      (f"GWOU{l}", W[f"GWOU{l}"]),
                      (f"GBOU{l}", W[f"GBOU{l}"].reshape(128, 1)),
                      (f"GWOI{l}", W[f"GWOI{l}"]),
                      (f"GBOI{l}", W[f"GBOI{l}"].reshape(128, 1))):
            names.append(nm)
            arrs.append(rep(a))
    names += ["WLIN", "BLIN"]
    arrs += [rep(W["WLIN"]), rep(W["BLIN"].reshape(128, 1))]
    put = jax.device_put([np.ascontiguousarray(a) for a in arrs],
                         [shard] * len(arrs))
    up = dict(zip(names, put))
    args1 = dict(xuT=up["xuT"], xiT=up["xiT"], WINU=up["WINU"], BINU=up["BINU"],
                 WINI=up["WINI"], BINI=up["BINI"])
    TAB = up["TAB"]
    wb = up
    wlin = up["WLIN"]
    blin = up["BLIN"]
    _t("uploads")
    if futs is not None:
        _cache[key] = (futs["p1"].result(), futs["p2"].result(),
                       futs["p3"].result(), futs["p4"].result(),
                       *futs["glue"].result())
    p1, p2, p3, p4, agj, _zeros = _cache[key]
    _t("program builds+compiles joined")

    r1 = p1(mesh, _zeros, **args1, WBU=wb["WBU0"], BBU=wb["BBU0"],
            WBI=wb["WBI0"], BBI=wb["BBI0"])
    Y, XT = r1["Y"], r1["XT"]
    _t("P1 dispatch")
    Yfull = agj(Y)
    _t("AG1 dispatch")
    OUT = p2(mesh, _zeros, Yfull=Yfull, Yloc=Y, TAB=TAB)["OUT"]
    _t("P2a dispatch")
    r3 = p3(mesh, _zeros, OUT=OUT, XTP=XT,
            GWOU=wb["GWOU0"], GBOU=wb["GBOU0"], GWOI=wb["GWOI0"], GBOI=wb["GBOI0"],
            WBU=wb["WBU1"], BBU=wb["BBU1"], WBI=wb["WBI1"], BBI=wb["BBI1"])
    Y2, XT2 = r3["Y"], r3["XT2"]
    _t("P3 dispatch")
    Yfull2 = agj(Y2)
    OUT2 = p2(mesh, _zeros, Yfull=Yfull2, Yloc=Y2, TAB=TAB)["OUT"]
    _t("AG2+P2b dispatch")
    r4 = p4(mesh, _zeros, OUT=OUT2, XTP=XT2,
            GWOU=wb["GWOU1"], GBOU=wb["GBOU1"], GWOI=wb["GWOI1"], GBOI=wb["GBOI1"],
            WLIN=wlin, BLIN=blin)
    _t("P4 dispatch")
    FINT = np.asarray(r4["FINT"]).reshape(NC, 64, RALL)
    _t("FINT download/block")
    dt_ns = int((time.time() - t0) * 1e9)
    _LAST_HW_NS = dt_ns
    _HW_NS_TOTAL += dt_ns

    out = _assemble(pre, FINT)
    _t("assemble")
    return out
